# revision 81
# baseline (speedup 1.0000x reference)
"""Trainium2 Bass kernel for nn_Encoder_Decoder_Wrapper (conv encoder -> NTM step -> conv decoder).

Sharding: pure data parallel, batch 64 -> 8 cores x 8 samples; weights
replicated.  Per core, samples run in 4 pairs of 2 so every 64-channel conv
is a K=128/M=128 block-diagonal matmul (2 samples packed in contraction and
output partitions).

conv0 patch staging: 9 tap-shifted image copies per pair, partition order
r = dy + 3s + 6dx, so one 96KB HBM DMA per (pair, dy) fills a stride-3
partition set with the (s, dx) tap shifts expressed as linear source axes
(12 DMAs total).  dx=0 row-wrap elements are killed by an aligned DVE
strided memset, dx=2 by an HWDGE zero-copy DMA (SWDGE would generate its
384 4-byte descriptors serially on Q7, ~12us).  Pair 3 sits at partitions
64-81 of a second tile to use the odd SDMA-engine set.  conv0 stays fp32r
(its rhs streams straight from the f32 input).

NTM step reduced via its constant initial state: reads0 = h0 = c0 = 0, so
only out = clip(h @ w_out[:256] + b_out) with h = sig(o)*tanh(sig(i)*tanh(g));
the dropped ~1e-6 read vectors change the output by ~2e-4 relative.

Decoder: conv3/conv4 consume 2x nearest-neighbor-upsampled inputs, so
conv(up2(X)) is FOLDED into 4 output-phase convolutions with 2x2 kernels
over the un-upsampled X (phase (a,b) kernel (ey,ex) = sum of W[dy,dx] over
dy in GRP[a][ey], dx in GRP[b][ex], summed in PSUM by the weight-prep
transposes): 2.25x fewer matmul columns and no upsample staging.  Each
32-row conv4 block stores as one 1MB single-partition-axis DMA as soon as
its 4 phases evict, fully overlapping the 8MB writeback.

conv1 and the decoder run in bfloat16 (weights + staged activations, fp32
PSUM accumulation): fp32r matmuls execute two-pass (fp32_mode=LOW_HIGH)
with a ~190ns fused 4-byte weight load that dominates short tiles; bf16
streams single-pass with half-size weight loads and doubles DVE eviction
throughput.  Total rel err ~6e-3 vs the 2e-2 budget.

Dummy N=512 matmul bursts ("warm") keep the PE HAM clock gate at 2.4GHz
through the patch-staging window and the NTM serial section: the gate
falls to 1.2GHz after ~3.4us idle and needs ~3.4us of dense busy to
recover, so each cold dip costs ~10us.  Bursts must be emitted BEFORE any
instruction that blocks on a slow load (engine queues are FIFO).
"""

import os
import sys

sys.path.insert(0, "/opt/trn_rl_repo")
os.environ.setdefault("MYCRO_LOCAL_CACHE", "1")

import numpy as np

import concourse.bass as bass
import concourse.bacc as bacc
import concourse.mybir as mybir
import concourse.tile as tile
from concourse.masks import make_identity

F32 = mybir.dt.float32
F32R = mybir.dt.float32r
B16 = mybir.dt.bfloat16
AF = mybir.ActivationFunctionType
ALU = mybir.AluOpType

TAPS = [(dy, dx) for dy in range(3) for dx in range(3)]
CLIP = 20.0

N_CORES = 8
B_CORE = 8          # samples per core
NPAIR = B_CORE // 2

PATW = 4240         # per-partition conv0 patch buffer (elements)
PBASE = 65          # read-window base offset: window j in [PBASE, PBASE+4096)


def build_nc(debug=False):
    nc = bacc.Bacc(None, target_bir_lowering=False)

    inp = nc.dram_tensor("inputs", [B_CORE, 1, 64, 64], F32R, kind="ExternalInput")
    wc0 = nc.dram_tensor("w_conv0", [64, 1, 3, 3], F32, kind="ExternalInput")
    bc0 = nc.dram_tensor("b_conv0", [64], F32, kind="ExternalInput")
    wc1 = nc.dram_tensor("w_conv1", [64, 64, 3, 3], F32, kind="ExternalInput")
    bc1 = nc.dram_tensor("b_conv1", [64], F32, kind="ExternalInput")
    wen = nc.dram_tensor("w_enc", [1, 64, 3, 3], F32R, kind="ExternalInput")
    ben = nc.dram_tensor("b_enc", [1], F32, kind="ExternalInput")
    wc2 = nc.dram_tensor("w_conv2", [64, 1, 3, 3], F32, kind="ExternalInput")
    bc2 = nc.dram_tensor("b_conv2", [64], F32, kind="ExternalInput")
    wc3 = nc.dram_tensor("w_conv3", [64, 64, 3, 3], F32, kind="ExternalInput")
    bc3 = nc.dram_tensor("b_conv3", [64], F32, kind="ExternalInput")
    wc4 = nc.dram_tensor("w_conv4", [64, 64, 3, 3], F32, kind="ExternalInput")
    bc4 = nc.dram_tensor("b_conv4", [64], F32, kind="ExternalInput")
    wlx = nc.dram_tensor("w_lstm_x", [1024, 1024], F32R, kind="ExternalInput")
    bls = nc.dram_tensor("b_lstm", [1024], F32, kind="ExternalInput")
    wou = nc.dram_tensor("w_out", [1024, 256], F32R, kind="ExternalInput")
    bou = nc.dram_tensor("b_out", [256], F32R, kind="ExternalInput")
    out = nc.dram_tensor("out", [B_CORE, 64, 64, 64], F32, kind="ExternalOutput")

    dbg = {}
    if debug:
        for name, shape, dt in [
            ("dbg_h", [128, 2, 8], F32R),
            ("dbg_clip", [B_CORE, 16, 16], F32R),
            ("dbg_x", [B_CORE, 16, 16], F32),
            ("dbg_pat", [18, PATW], F32R),
            ("dbg_ct2", [128, 128], F32R),
            ("dbg_c1in", [128, 34, 34], F32R),
            ("dbg_pc2", [128, 684], F32R),
            ("dbg_ctc2", [128, 128], F32R),
        ]:
            dbg[name] = nc.dram_tensor(name, shape, dt, kind="ExternalOutput")

    with tile.TileContext(nc) as tc:
        with (
            tc.tile_pool(name="const", bufs=1) as const,
            tc.tile_pool(name="work", bufs=1) as work,
            tc.tile_pool(name="dbl", bufs=2) as dbl,
            tc.tile_pool(name="trip", bufs=3) as trip,
            tc.tile_pool(name="tri3", bufs=3) as tri3,
            tc.tile_pool(name="quad", bufs=4) as quad,
            tc.tile_pool(name="c3p", bufs=3) as c3p,
            tc.tile_pool(name="out2", bufs=2) as out2,
            tc.tile_pool(name="psmm", bufs=6, space="PSUM") as psmm,
            tc.tile_pool(name="psc3", bufs=2, space="PSUM") as psc3,
        ):
            dmaeng = [nc.sync, nc.gpsimd, nc.scalar]
            wrap_zero = bool(int(os.environ.get("KWRAP", "1")))

            # ---------------- conv0 patch buffer + pad memsets --------------
            # pairs 0-2 at base partitions 0/32/64; pair 3 (base partition
            # 96 is not a legal matmul operand base) lives at base 0 of a
            # second column range.
            # pair 3 lives at partitions 64-81 of a second tile so its patch
            # DMAs land on the ODD SDMA-engine set (partitions 64+) and
            # balance against pairs 0/1 on the even set.
            pat = const.tile([128, PATW], F32R, tag="pat")
            pat3 = const.tile([128, PATW], F32R, tag="pat3")
            patT = pat[:].tensor
            patO = pat[:].offset
            pat3T = pat3[:].tensor
            pat3O = pat3[:].offset + 64 * PATW
            # top/bottom row pads of the read window (shared by all taps);
            # emitted BEFORE the patch DMAs so interiors get overwritten.
            nc.vector.memset(pat[:, PBASE : PBASE + 65].bitcast(F32), 0.0)
            nc.vector.memset(pat[:, PBASE + 4031 : PBASE + 4096].bitcast(F32), 0.0)
            nc.vector.memset(pat3[:, PBASE : PBASE + 65].bitcast(F32), 0.0)
            nc.vector.memset(pat3[:, PBASE + 4031 : PBASE + 4096].bitcast(F32), 0.0)

            def pat_loc(p):
                if p < 3:
                    return patT, patO + 32 * p * PATW, pat
                return pat3T, pat3O, pat3

            # ---------------- conv0 patch DMAs: issued FIRST ----------------
            # Patch partition order r = 6dy + 3s + dx.  For a fixed dy all 6
            # partitions share the same trimmed dst window [d0, d0+4094)
            # (d0 = 66 - 64(dy-1)), and the (s, dx) tap shifts become LINEAR
            # source axes [[4096, 2], [1, 3]], so ONE 96KB HBM DMA fills the
            # whole dy-group: 3 DMAs per pair instead of 9.  (DMA AP shapes
            # need not match across sides; only the final contiguous dim
            # must.)  The few corner elements the common trim loses read as
            # the zero pad: ~4 real pixels per sample, negligible.
            pengs = [nc.sync, nc.scalar]
            if wrap_zero:
                zsrc = const.tile([128, 64], F32R, tag="zsrc")
                nc.vector.memset(zsrc[:].bitcast(F32), 0.0)

            # Partition order r = dy + 3s + 6dx: each dy-group is the
            # stride-3 set {dy, dy+3, .., dy+15} (spans ~4 SDMA engines
            # instead of 2), dx=0 is partitions 0-5 (aligned -> DVE memset
            # wrap-zero) and dx=2 is partitions 12-17 (one gpsimd zero-DMA).
            def patch_pair(p):
                pT, pO, _ = pat_loc(p)
                for dy in range(3):
                    d0 = 66 - 64 * (dy - 1)
                    (nc.sync if p < 2 else nc.gpsimd).dma_start(
                        out=bass.AP(
                            tensor=pT,
                            offset=pO + dy * PATW + d0,
                            ap=[[3 * PATW, 6], [1, 4094]],
                        ),
                        in_=bass.AP(
                            tensor=inp[:].tensor,
                            offset=2 * p * 4096,
                            ap=[[1, 3], [4096, 2], [1, 4094]],
                        ),
                    )

            def wrap_pair(p):
                # wrap-element zeroing, emitted AFTER all patch DMAs so the
                # completion waits never block further patch issues:
                #  dx=0: X=0 reads hit j = PBASE+64Y    -> zero [PBASE::64]
                #  dx=2: X=63 reads hit j = PBASE+63+64Y -> zero [PBASE+63::64]
                pT, pO, _ = pat_loc(p)
                nc.vector.memset(
                    bass.AP(
                        tensor=pT,
                        offset=pO + PBASE,
                        ap=[[PATW, 6], [64, 64]],
                    ).bitcast(F32),
                    0.0,
                )
                # partition base 12 is not DVE/gpsimd-writable (32-alignment
                # rule) -> zero-copy DMA for the dx=2 group.  MUST be HWDGE
                # (sync/scalar): SWDGE generates the 384 4B descriptors
                # serially on Q7 (~12us); HWDGE does it in RTL and spreads
                # them across all 16 SDMA slots (~1us).
                nc.sync.dma_start(
                    out=bass.AP(
                        tensor=pT,
                        offset=pO + 12 * PATW + PBASE + 63,
                        ap=[[PATW, 6], [64, 64]],
                    ),
                    in_=bass.AP(
                        tensor=zsrc[:].tensor,
                        offset=zsrc[:].offset,
                        ap=[[64, 6], [1, 64]],
                    ),
                )

            patch_pair(0)

            # stg2: conv2 padded staging rows (one partition per sample)
            stg2 = const.tile([8, 21, 19], F32R, tag="stg2")
            nc.vector.memset(stg2[:].bitcast(F32), 0.0)

            # ---------------- weight staging DMAs ---------------------------
            # s9x[c, 2q+s] (q = 3dx+dy) holds w_conv0[c%64, dy, dx] in the
            # (c<64) == (s==0) half, zero elsewhere; one PE transpose then
            # yields the conv0 lhsT block directly.  Staged via a t-order
            # load + 6 small strided gathers (all partition-step-1 APs).
            # s9t + gathers go on HWDGE (sync/scalar): as many-small-
            # descriptor DMAs they would serialize SWDGE's Q7 descriptor
            # generation for ~15us and gate ct2_c0 (and so conv0).
            s9t = const.tile([64, 9], F32, tag="s9t")
            nc.scalar.dma_start(out=s9t[:], in_=wc0[:].rearrange("a b c d -> a (b c d)"))
            s9x = const.tile([128, 18], F32, tag="s9x")
            nc.vector.memset(s9x[:], 0.0)
            for s in range(2):
                for dx in range(3):
                    nc.scalar.dma_start(
                        out=bass.AP(
                            tensor=s9x[:].tensor,
                            offset=s9x[:].offset + s * (64 * 18) + 3 * s + 6 * dx,
                            ap=[[18, 64], [1, 3]],
                        ),
                        in_=bass.AP(
                            tensor=s9t[:].tensor,
                            offset=s9t[:].offset + dx,
                            ap=[[9, 64], [3, 3]],
                        ),
                    )
            def load_wsrc(wdram):
                wsrc = dbl.tile([64, 576], F32, tag="wsrc")
                nc.gpsimd.dma_start(
                    out=wsrc[:], in_=wdram[:].rearrange("a b c d -> a (b c d)")
                )
                return wsrc

            # ---------------- conv biases (bt0/bt1 needed by the encoder
            # evictions; the rest load after the patch DMAs) ---------------
            def bias128(dram_b, tag, eng):
                bt = const.tile([128, 1], F32, tag=tag)
                eng.dma_start(out=bt[0:64, :], in_=dram_b[:].unsqueeze(1))
                eng.dma_start(out=bt[64:128, :], in_=dram_b[:].unsqueeze(1))
                return bt

            patch_pair(1)
            patch_pair(2)
            patch_pair(3)
            bt0 = bias128(bc0, "bt0", nc.gpsimd)
            bt1 = bias128(bc1, "bt1", nc.gpsimd)
            bt0s = const.tile([128, 1], F32, tag="bt0s")
            nc.vector.tensor_scalar_mul(bt0s[:], bt0[:], 0.25)
            bt1s = const.tile([128, 1], F32, tag="bt1s")
            nc.vector.tensor_scalar_mul(bt1s[:], bt1[:], 0.25)
            # pair-0's wrap zero issues right after the s9x gathers so
            # conv0 can start at ~16us; later pairs' zeros interleave with
            # the remaining weight loads.
            if wrap_zero:
                wrap_pair(0)
                wrap_pair(1)
            wsrc_c1 = load_wsrc(wc1)
            if wrap_zero:
                wrap_pair(2)
                wrap_pair(3)
            # ---------------- identity (for PE transposes) ------------------
            ident = const.tile([128, 128], F32, tag="ident")
            make_identity(nc, ident)

            # ---------------- PE warmup -------------------------------------
            # The HAM clock gate holds the PE at 1.2GHz until it has been
            # busy for a full 3.4us window.  While the patch DMAs stage,
            # keep the PE streaming dummy matmuls so the encoder starts at
            # 2.4GHz instead of half clock.
            wgarb = const.tile([128, 512], F32R, tag="wgarb")
            nc.vector.memset(wgarb[:].bitcast(F32), 0.0)

            def warm(n):
                # N=512 dummies: 213ns of guaranteed-dense PE busy each, so
                # a burst of ~16 always covers the 3.4us HAM re-warm window
                for _ in range(n):
                    pwu = psmm.tile([64, 512], F32, tag="mm")
                    nc.tensor.matmul(
                        pwu[:], wgarb[:, 0:64], wgarb[:], start=True, stop=True
                    )

            warm(62)

            # ---------------- 1ch conv weights ------------------------------
            # staged t-order [9, 64] via PE transpose, then scattered to the
            # block-diagonal replicated lhsT tiles with one remap DMA each.
            ct2_c0 = const.tile([128, 128], F32R, tag="ct2_c0")
            nc.vector.memset(ct2_c0[:].bitcast(F32), 0.0)
            ct2_c2 = const.tile([128, 128], F32R, tag="ct2_c2")
            nc.vector.memset(ct2_c2[:].bitcast(F32), 0.0)

            p9 = psmm.tile([18, 128], F32, tag="mm")
            nc.tensor.transpose(p9[:], s9x[:], ident[0:128, 0:128])
            nc.scalar.activation(
                ct2_c0[0:18, :], p9[:], AF.Copy, bias=0.0, scale=1.0
            )
            for p in (1, 2):
                nc.sync.dma_start(
                    out=ct2_c0[32 * p : 32 * p + 18, :],
                    in_=ct2_c0[0:18, :],
                )


            # ---------------- 64ch conv weights -> block-diag lhsT ----------
            wtap = {}

            def build_wtap(name, wsrc):
                wt = const.tile([128, 9, 128], B16, tag=f"wtap_{name}")
                nc.vector.memset(wt[:], 0.0)
                for t in range(9):
                    pw = psmm.tile([64, 64], F32, tag="mm")
                    nc.tensor.transpose(pw[:], wsrc[:, t::9], ident[0:64, 0:64])
                    nc.scalar.activation(
                        wt[0:64, t, 0:64], pw[:], AF.Copy, bias=0.0, scale=1.0
                    )
                nc.sync.dma_start(out=wt[64:128, :, 64:128], in_=wt[0:64, :, 0:64])
                wtap[name] = wt

            build_wtap("c1", wsrc_c1)

            # enc conv weights (64ci -> 1co): one gather DMA, no scaling.
            encT = const.tile([128, 9, 2], F32R, tag="encT")
            nc.vector.memset(encT[:].bitcast(F32), 0.0)
            # out (c+64s)*18 + 2t + s  <-  wen flat c*9 + t  (one DMA per s)
            for s in range(2):
                nc.scalar.dma_start(
                    out=bass.AP(
                        tensor=encT[:].tensor,
                        offset=encT[:].offset + s * (64 * 18 + 1),
                        ap=[[18, 64], [2, 9]],
                    ),
                    in_=bass.AP(
                        tensor=wen[:].tensor,
                        offset=0,
                        ap=[[9, 64], [1, 9]],
                    ),
                )

            bte = const.tile([2, 1], F32, tag="bte")
            nc.sync.dma_start(
                out=bte[:],
                in_=bass.AP(tensor=ben[:].tensor, offset=0, ap=[[0, 2], [1, 1]]),
            )

            xstage = const.tile([8, 16, 16], F32, tag="xstage")

            # ================ encoder: interleaved over 4 sample pairs ======
            c1in_l = [None] * NPAIR
            ein_l = [None] * NPAIR

            def conv0_pair(p):
                c1in = tri3.tile([128, 34, 34], B16, tag="c1in")
                nc.gpsimd.memset(c1in[:, 0:1, :], 0.0)
                nc.gpsimd.memset(c1in[:, 33:34, :], 0.0)
                nc.gpsimd.memset(c1in[:, 1:33, 0:1], 0.0)
                nc.gpsimd.memset(c1in[:, 1:33, 33:34], 0.0)
                base = 32 * p if p < 3 else 64
                _, _, ptile = pat_loc(p)
                for n in range(8):
                    ps = psmm.tile([128, 4, 2, 32, 2], F32, tag="mm")
                    nc.tensor.matmul(
                        ps[:].rearrange("p a b c d -> p (a b c d)"),
                        ct2_c0[base : base + 18, :],
                        ptile[base : base + 18, PBASE + 512 * n : PBASE + 512 * (n + 1)],
                        start=True,
                        stop=True,
                    )
                    ct0 = trip.tile([128, 4, 2, 32, 2], F32, tag="ct0")
                    nc.scalar.activation(ct0[:], ps[:], AF.Relu, bias=bt0s, scale=0.25)
                    tcol = tri3.tile([128, 4, 2, 32], F32, tag="tcol")
                    nc.vector.tensor_add(
                        tcol[:], ct0[:, :, :, :, 0], ct0[:, :, :, :, 1]
                    )
                    nc.vector.tensor_add(
                        c1in[:, 1 + 4 * n : 5 + 4 * n, 1:33],
                        tcol[:, :, 0, :],
                        tcol[:, :, 1, :],
                    )
                c1in_l[p] = c1in

            def conv1_pair(p):
                c1in = c1in_l[p]
                e_in = quad.tile([128, 18, 18], F32R, tag="e_in")
                nc.gpsimd.memset(e_in[:, 0:1, :].bitcast(F32), 0.0)
                nc.gpsimd.memset(e_in[:, 17:18, :].bitcast(F32), 0.0)
                nc.gpsimd.memset(e_in[:, 1:17, 0:1].bitcast(F32), 0.0)
                nc.gpsimd.memset(e_in[:, 1:17, 17:18].bitcast(F32), 0.0)
                for n in range(2):
                    ps = psmm.tile([128, 8, 2, 16, 2], F32, tag="mm")
                    for t, (dy, dx) in enumerate(TAPS):
                        nc.tensor.matmul(
                            ps[:],
                            wtap["c1"][:, t, :],
                            c1in[:, n * 16 + dy : n * 16 + dy + 16, dx : dx + 32],
                            start=(t == 0),
                            stop=(t == 8),
                        )
                    ct1 = trip.tile([128, 8, 2, 16, 2], F32, tag="ct1")
                    nc.scalar.activation(ct1[:], ps[:], AF.Relu, bias=bt1s, scale=0.25)
                    tc1 = tri3.tile([128, 8, 2, 16], F32, tag="tc1")
                    nc.vector.tensor_add(
                        tc1[:], ct1[:, :, :, :, 0], ct1[:, :, :, :, 1]
                    )
                    nc.vector.tensor_add(
                        e_in[:, 1 + 8 * n : 9 + 8 * n, 1:17],
                        tc1[:, :, 0, :],
                        tc1[:, :, 1, :],
                    )
                ein_l[p] = e_in

            def enc_pair(p):
                e_in = ein_l[p]
                pe = psmm.tile([2, 16, 16], F32, tag="mm")
                for t, (dy, dx) in enumerate(TAPS):
                    nc.tensor.matmul(
                        pe[:],
                        encT[:, t, :],
                        e_in[:, dy : dy + 16, dx : dx + 16],
                        start=(t == 0),
                        stop=(t == 8),
                    )
                estage = dbl.tile([2, 16, 16], F32, tag="estage")
                nc.scalar.activation(estage[:], pe[:], AF.Relu, bias=bte)
                nc.scalar.dma_start(out=xstage[2 * p : 2 * p + 2, :, :], in_=estage[:])

            conv0_pair(0)
            if debug:
                nc.sync.dma_start(out=dbg["dbg_pat"][:], in_=pat[0:18, :])
                nc.sync.dma_start(out=dbg["dbg_ct2"][:], in_=ct2_c0[:])
                nc.sync.dma_start(out=dbg["dbg_c1in"][:], in_=c1in_l[0][:])
            conv0_pair(1)
            warm(10)
            conv0_pair(2)
            conv1_pair(0)
            warm(1)
            enc_pair(0)
            conv1_pair(1)
            warm(1)
            conv0_pair(3)
            enc_pair(1)
            conv1_pair(2)
            warm(1)
            enc_pair(2)
            conv1_pair(3)
            warm(1)
            enc_pair(3)

            # ---------------- deferred weight staging (decoder + NTM): -----
            # emitted after the encoder so these HBM loads queue BEHIND the
            # conv0 patch DMAs and don't stall the pipeline start; they
            # drain during the encoder compute.
            # s9y[c, 9s+t] = w_conv2[c%64, t] in the matching half (t-order).
            s9y = const.tile([128, 18], F32, tag="s9y")
            nc.vector.memset(s9y[:], 0.0)
            for s in range(2):
                nc.scalar.dma_start(
                    out=bass.AP(
                        tensor=s9y[:].tensor,
                        offset=s9y[:].offset + s * (64 * 18 + 9),
                        ap=[[18, 64], [1, 9]],
                    ),
                    in_=wc2[:].rearrange("a b c d -> a (b c d)"),
                )
            p9b = psmm.tile([18, 128], F32, tag="mm")
            nc.tensor.transpose(p9b[:], s9y[:], ident[0:128, 0:128])
            nc.scalar.activation(
                ct2_c2[0:18, :], p9b[:], AF.Copy, bias=0.0, scale=1.0
            )
            for p in (1, 2):
                nc.scalar.dma_start(
                    out=ct2_c2[32 * p : 32 * p + 18, :],
                    in_=ct2_c2[0:18, :],
                )
            bt2 = bias128(bc2, "bt2", nc.scalar)
            bt3 = bias128(bc3, "bt3", nc.sync)
            bt4 = bias128(bc4, "bt4", nc.scalar)

            # NTM weights: w_lstm_x rows 0:256, gate cols i/g/o
            wx = const.tile([128, 2, 768], F32R, tag="wx")
            for kt in range(2):
                nc.gpsimd.dma_start(
                    out=wx[:, kt, 0:256],
                    in_=wlx[kt * 128 : (kt + 1) * 128, 0:256],
                )
                nc.gpsimd.dma_start(
                    out=wx[:, kt, 256:768],
                    in_=wlx[kt * 128 : (kt + 1) * 128, 512:1024],
                )
            bigo = const.tile([128, 6], F32, tag="bigo")
            # cols (2j+h2): j in (i,g,o) -> b_lstm[0:256] and b_lstm[512:1024]
            nc.sync.dma_start(
                out=bass.AP(tensor=bigo[:].tensor, offset=bigo[:].offset,
                            ap=[[6, 128], [1, 2]]),
                in_=bass.AP(tensor=bls[:].tensor, offset=0,
                            ap=[[1, 128], [128, 2]]),
            )
            nc.sync.dma_start(
                out=bass.AP(tensor=bigo[:].tensor, offset=bigo[:].offset + 2,
                            ap=[[6, 128], [1, 4]]),
                in_=bass.AP(tensor=bls[:].tensor, offset=512,
                            ap=[[1, 128], [128, 4]]),
            )
            # w_out rows 0:256 (h part) + bias row
            wo = const.tile([128, 2, 256], F32R, tag="wo")
            nc.gpsimd.dma_start(out=wo[:, 0, :], in_=wou[0:128, :])
            nc.gpsimd.dma_start(out=wo[:, 1, :], in_=wou[128:256, :])
            rhs_b = const.tile([1, 256], F32R, tag="rhs_b")
            nc.scalar.dma_start(out=rhs_b[:], in_=bou[:].unsqueeze(0))
            ones1 = const.tile([1, 8], F32R, tag="ones1")
            nc.vector.memset(ones1[:].bitcast(F32), 1.0)

            # deferred weight prep: FOLDED decoder kernels (fills the PE
            # bubble while the NTM chain runs).  conv3/conv4 consume a 2x
            # nearest-neighbor-upsampled input, so conv(up2(X)) collapses
            # into 4 output-phase convolutions with 2x2 kernels over the
            # un-upsampled X: phase (a,b) kernel (ey,ex) = sum of W[dy,dx]
            # over dy in GRP[a][ey], dx in GRP[b][ex].  The tap sums are
            # accumulated directly in PSUM by the transposes.
            GRP = (((0,), (1, 2)), ((0, 1), (2,)))
            wfold = {}

            def build_wfold(name, wsrc):
                wf = const.tile([128, 16, 128], B16, tag=f"wfold_{name}")
                nc.vector.memset(wf[:], 0.0)
                for a in range(2):
                    for b in range(2):
                        for ey in range(2):
                            for ex in range(2):
                                k = 8 * a + 4 * b + 2 * ey + ex
                                taps = [
                                    3 * dy + dx
                                    for dy in GRP[a][ey]
                                    for dx in GRP[b][ex]
                                ]
                                pw = psmm.tile([64, 64], F32, tag="mm")
                                for i, t in enumerate(taps):
                                    nc.tensor.matmul(
                                        pw[:],
                                        wsrc[:, t::9],
                                        ident[0:64, 0:64],
                                        is_transpose=True,
                                        start=(i == 0),
                                        stop=(i == len(taps) - 1),
                                    )
                                nc.scalar.activation(
                                    wf[0:64, k, 0:64], pw[:], AF.Copy,
                                    bias=0.0, scale=1.0,
                                )
                nc.sync.dma_start(out=wf[64:128, :, 64:128], in_=wf[0:64, :, 0:64])
                wfold[name] = wf

            wsrc_c3 = load_wsrc(wc3)
            build_wfold("c3", wsrc_c3)
            wsrc_c4 = load_wsrc(wc4)
            build_wfold("c4", wsrc_c4)
            # bridge the gap while the enc evictions land in xstage
            warm(5)

            # ================ NTM step (all 8 samples at once) ==============
            if debug:
                nc.sync.dma_start(out=dbg["dbg_x"][:], in_=xstage[:])
            # x^T k-tiles via PE transpose
            xT = work.tile([128, 2, 8], F32R, tag="xT")
            for kt in range(2):
                pxt = psmm.tile([128, 8], F32, tag="mm")
                nc.tensor.transpose(
                    pxt[:],
                    xstage[:].rearrange("p a b -> p (a b)")[:, kt * 128 : kt * 128 + 128],
                    ident[0:8, 0:8],
                )
                nc.scalar.activation(xT[:, kt, :], pxt[:], AF.Copy, bias=0.0, scale=1.0)
            # z = x @ Wx + b for gates i, g, o; h = sig(o) * tanh(sig(i)*tanh(g))
            zps = psmm.tile([128, 6, 8], F32, tag="mm")
            for j in range(3):
                for h2 in range(2):
                    for kt in range(2):
                        nc.tensor.matmul(
                            zps[:, 2 * j + h2, :],
                            wx[:, kt, j * 256 + h2 * 128 : j * 256 + h2 * 128 + 128],
                            xT[:, kt, :],
                            start=(kt == 0),
                            stop=(kt == 1),
                        )
            zb = work.tile([128, 6, 8], F32, tag="zb")
            bigo_b = bass.AP(
                tensor=bigo[:].tensor, offset=bigo[:].offset,
                ap=[list(d) for d in bigo[:].ap] + [[0, 8]],
            )
            nc.vector.tensor_tensor(zb[:], zps[:], bigo_b, op=ALU.add)
            si = work.tile([128, 2, 8], F32, tag="gate0")
            nc.scalar.activation(si[:], zb[:, 0:2, :], AF.Sigmoid, bias=0.0)
            tg = work.tile([128, 2, 8], F32, tag="gate1")
            nc.scalar.activation(tg[:], zb[:, 2:4, :], AF.Tanh, bias=0.0)
            so = work.tile([128, 2, 8], F32, tag="gate2")
            nc.scalar.activation(so[:], zb[:, 4:6, :], AF.Sigmoid, bias=0.0)
            ctile = work.tile([128, 2, 8], F32, tag="ctile")
            nc.vector.tensor_mul(ctile[:], si[:], tg[:])
            tct = work.tile([128, 2, 8], F32, tag="tct")
            nc.scalar.activation(tct[:], ctile[:], AF.Tanh, bias=0.0)
            h = work.tile([128, 2, 8], F32R, tag="h")
            nc.vector.tensor_mul(h[:], so[:], tct[:])
            if debug:
                nc.sync.dma_start(out=dbg["dbg_h"][:], in_=h[:])
            # out = clip(h @ w_out[:256] + b_out)  (reads contribution dropped)
            pout = psmm.tile([8, 16, 16], F32, tag="mm")
            for kt in range(2):
                nc.tensor.matmul(
                    pout[:].rearrange("p a b -> p (a b)"),
                    h[:, kt, :],
                    wo[:, kt, :],
                    start=(kt == 0),
                    stop=False,
                )
            nc.tensor.matmul(
                pout[:].rearrange("p a b -> p (a b)"),
                ones1[:],
                rhs_b[:],
                start=False,
                stop=True,
            )
            nc.vector.tensor_scalar(
                stg2[:, 1:17, 1:17], pout[:], -CLIP, CLIP, ALU.max, ALU.min
            )
            # keep the PE clock warm while the NTM result fans out through
            # stg2 -> pc2 staging DMAs: the HAM gate needs a DENSE ~3.4us
            # busy window to hold/raise 2.4GHz, and a cold decoder start
            # costs ~15us.
            warm(18)
            if debug:
                nc.sync.dma_start(out=dbg["dbg_clip"][:], in_=stg2[:, 1:17, 1:17])

            # ================ decoder: stage-major over 4 pairs =============
            # conv2 patches for all pairs in one merged tile + one DMA.
            # partition r = 32p + 9s + 3dy + dx via the overlapping stride-1
            # dx trick (reads stg2 shifted by 0/1/2 columns).
            pc2 = const.tile([128, 684], F32R, tag="pc2")
            for p in range(NPAIR):
                base = 32 * p if p < 3 else 0
                c0 = 0 if p < 3 else 342
                for s in range(2):
                    for dy in range(3):
                        eng = dmaeng[(2 * p + s + dy) % 3]
                        eng.dma_start(
                            out=bass.AP(
                                tensor=pc2[:].tensor,
                                offset=pc2[:].offset
                                + (base + 9 * s + 3 * dy) * 684 + c0,
                                ap=[[684, 3], [1, 341]],
                            ),
                            in_=bass.AP(
                                tensor=stg2[:].tensor,
                                offset=stg2[:].offset + (2 * p + s) * 399 + dy * 19,
                                ap=[[399, 1], [1, 3], [1, 341]],
                            ),
                        )

            # --- conv2 all pairs -> padded stage S2 (reuses the quad pool
            # slots freed by the enc pairs; 1 eviction per pair instead of
            # the old 4 upsample writes)
            c2s_l = []
            for p in range(NPAIR):
                base = 32 * p if p < 3 else 0
                c0 = 0 if p < 3 else 342
                ps2 = psmm.tile([128, 16, 16], F32, tag="mm")
                nc.tensor.matmul(
                    ps2[:],
                    ct2_c2[base : base + 18, :],
                    pc2[base : base + 18, c0 : c0 + 342]
                    .rearrange("p (a b) -> p a b", a=18)[:, 0:16, 0:16],
                    start=True,
                    stop=True,
                )
                S2 = quad.tile([128, 18, 18], B16, tag="c2s")
                nc.gpsimd.memset(S2[:, 0:1, :], 0.0)
                nc.gpsimd.memset(S2[:, 17:18, :], 0.0)
                nc.gpsimd.memset(S2[:, 1:17, 0:1], 0.0)
                nc.gpsimd.memset(S2[:, 1:17, 17:18], 0.0)
                nc.scalar.activation(S2[:, 1:17, 1:17], ps2[:], AF.Relu, bias=bt2)
                c2s_l.append(S2)
            if debug:
                nc.sync.dma_start(out=dbg["dbg_pc2"][:], in_=pc2[:])
                nc.sync.dma_start(out=dbg["dbg_ctc2"][:], in_=ct2_c2[:])

            # --- conv3 folded: 4 output-phase 2x2 convs over S2 -> S3
            # (34x34 padded, UN-upsampled conv3 output)
            c3s_l = [None] * NPAIR

            def conv3_pair(p):
                S2 = c2s_l[p]
                S3 = c3p.tile([128, 34, 34], B16, tag="c3s")
                nc.gpsimd.memset(S3[:, 0:1, :], 0.0)
                nc.gpsimd.memset(S3[:, 33:34, :], 0.0)
                nc.gpsimd.memset(S3[:, 1:33, 0:1], 0.0)
                nc.gpsimd.memset(S3[:, 1:33, 33:34], 0.0)
                S3v = S3[:].rearrange(
                    "p (ri ra) (ci cb) -> p ri ra ci cb", ra=2, cb=2
                )
                for a in range(2):
                    for b in range(2):
                        ps = psc3.tile([128, 16, 16], F32, tag="mm3")
                        i = 0
                        for ey in range(2):
                            for ex in range(2):
                                k = 8 * a + 4 * b + 2 * ey + ex
                                nc.tensor.matmul(
                                    ps[:],
                                    wfold["c3"][:, k, :],
                                    S2[:, a + ey : a + ey + 16,
                                       b + ex : b + ex + 16],
                                    start=(i == 0),
                                    stop=(i == 3),
                                )
                                i += 1
                        # out row 1+2r+a, col 1+2c+b in S3
                        rs = slice(0, 16) if a == 0 else slice(1, 17)
                        cs = slice(0, 16) if b == 0 else slice(1, 17)
                        dst = S3v[:, rs, 1 - a, cs, 1 - b]
                        if (a + b) % 2 == 0:
                            nc.scalar.activation(dst, ps[:], AF.Relu, bias=bt3)
                        else:
                            nc.vector.tensor_scalar(
                                dst, ps[:], bt3[:], 0.0, ALU.add, ALU.max
                            )
                c3s_l[p] = S3

            # --- conv4 folded: phases over S3; each 32-row block stores as
            # one 1MB DMA as soon as its 4 phases are evicted.
            store_rot = [nc.gpsimd, nc.sync]

            def conv4_pair(p):
                S3 = c3s_l[p]
                c4out = out2.tile([128, 64, 64], F32, tag="c4out")
                c4v = c4out[:].rearrange(
                    "p (ri ra) (ci cb) -> p ri ra ci cb", ra=2, cb=2
                )
                for h in range(2):
                    for a in range(2):
                        for b in range(2):
                            ps = psmm.tile([128, 16, 32], F32, tag="mm")
                            i = 0
                            for ey in range(2):
                                for ex in range(2):
                                    k = 8 * a + 4 * b + 2 * ey + ex
                                    nc.tensor.matmul(
                                        ps[:],
                                        wfold["c4"][:, k, :],
                                        S3[:, 16 * h + a + ey : 16 * h + a + ey + 16,
                                           b + ex : b + ex + 32],
                                        start=(i == 0),
                                        stop=(i == 3),
                                    )
                                    i += 1
                            dst = c4v[:, 16 * h : 16 * h + 16, a, 0:32, b]
                            if (a + b) % 2 == 0:
                                nc.scalar.activation(dst, ps[:], AF.Relu, bias=bt4)
                            else:
                                nc.vector.tensor_scalar(
                                    dst, ps[:], bt4[:], 0.0, ALU.add, ALU.max
                                )
                    store_rot[(2 * p + h) % 2].dma_start(
                        out=bass.AP(
                            tensor=out[:].tensor,
                            offset=2 * p * 262144 + 2048 * h,
                            ap=[[4096, 128], [1, 2048]],
                        ),
                        in_=bass.AP(
                            tensor=c4out[:].tensor,
                            offset=c4out[:].offset + 2048 * h,
                            ap=[[4096, 128], [1, 2048]],
                        ),
                    )

            conv3_pair(0)
            conv3_pair(1)
            conv4_pair(0)
            conv3_pair(2)
            conv4_pair(1)
            conv3_pair(3)
            conv4_pair(2)
            conv4_pair(3)

    nc.compile()
    return nc


_NC_CACHE = {}
LAST_RESULT = None

WEIGHT_NAMES = [
    "w_conv0", "b_conv0", "w_conv1", "b_conv1", "w_enc", "b_enc",
    "w_conv2", "b_conv2", "w_conv3", "b_conv3", "w_conv4", "b_conv4",
    "w_lstm_x", "b_lstm", "w_out", "b_out",
]


def kernel(**inputs):
    global LAST_RESULT
    from concourse.bass_utils import run_bass_kernel_spmd

    debug = bool(int(os.environ.get("KDEBUG", "0")))
    key = ("nc", debug)
    if key not in _NC_CACHE:
        _NC_CACHE[key] = build_nc(debug=debug)
    nc = _NC_CACHE[key]

    xs = np.ascontiguousarray(np.asarray(inputs["inputs"], dtype=np.float32))
    weights = {
        k: np.ascontiguousarray(np.asarray(inputs[k], dtype=np.float32))
        for k in WEIGHT_NAMES
    }
    in_maps = []
    for c in range(N_CORES):
        m = dict(weights)
        m["inputs"] = xs[c * B_CORE : (c + 1) * B_CORE]
        in_maps.append(m)

    res = run_bass_kernel_spmd(nc, in_maps, core_ids=list(range(N_CORES)))
    LAST_RESULT = res
    return np.concatenate([r["out"] for r in res.results], axis=0)


if __name__ == "__main__":
    nc = build_nc()
    print("built ok")



# revision 82
# speedup vs baseline: 1.0131x; 1.0131x over previous
"""Trainium2 Bass kernel for nn_Encoder_Decoder_Wrapper (conv encoder -> NTM step -> conv decoder).

Sharding: pure data parallel, batch 64 -> 8 cores x 8 samples; weights
replicated.  Per core, samples run in 4 pairs of 2 so every 64-channel conv
is a K=128/M=128 block-diagonal matmul (2 samples packed in contraction and
output partitions).

conv0 patch staging: 9 tap-shifted image copies per pair, partition order
r = dy + 3s + 6dx, so one 96KB HBM DMA per (pair, dy) fills a stride-3
partition set with the (s, dx) tap shifts expressed as linear source axes
(12 DMAs total).  dx=0 row-wrap elements are killed by an aligned DVE
strided memset, dx=2 by an HWDGE zero-copy DMA (SWDGE would generate its
384 4-byte descriptors serially on Q7, ~12us).  Pair 3 sits at partitions
64-81 of a second tile to use the odd SDMA-engine set.  conv0 stays fp32r
(its rhs streams straight from the f32 input).

NTM step reduced via its constant initial state: reads0 = h0 = c0 = 0, so
only out = clip(h @ w_out[:256] + b_out) with h = sig(o)*tanh(sig(i)*tanh(g));
the dropped ~1e-6 read vectors change the output by ~2e-4 relative.

Decoder: conv3/conv4 consume 2x nearest-neighbor-upsampled inputs, so
conv(up2(X)) is FOLDED into 4 output-phase convolutions with 2x2 kernels
over the un-upsampled X (phase (a,b) kernel (ey,ex) = sum of W[dy,dx] over
dy in GRP[a][ey], dx in GRP[b][ex], summed in PSUM by the weight-prep
transposes): 2.25x fewer matmul columns and no upsample staging.  Each
32-row conv4 block stores as one 1MB single-partition-axis DMA as soon as
its 4 phases evict, fully overlapping the 8MB writeback.

conv1 and the decoder run in bfloat16 (weights + staged activations, fp32
PSUM accumulation): fp32r matmuls execute two-pass (fp32_mode=LOW_HIGH)
with a ~190ns fused 4-byte weight load that dominates short tiles; bf16
streams single-pass with half-size weight loads and doubles DVE eviction
throughput.  Total rel err ~6e-3 vs the 2e-2 budget.

Dummy N=512 matmul bursts ("warm") keep the PE HAM clock gate at 2.4GHz
through the patch-staging window and the NTM serial section: the gate
falls to 1.2GHz after ~3.4us idle and needs ~3.4us of dense busy to
recover, so each cold dip costs ~10us.  Bursts must be emitted BEFORE any
instruction that blocks on a slow load (engine queues are FIFO).
"""

import os
import sys

sys.path.insert(0, "/opt/trn_rl_repo")
os.environ.setdefault("MYCRO_LOCAL_CACHE", "1")

import numpy as np

import concourse.bass as bass
import concourse.bacc as bacc
import concourse.mybir as mybir
import concourse.tile as tile
from concourse.masks import make_identity

F32 = mybir.dt.float32
F32R = mybir.dt.float32r
B16 = mybir.dt.bfloat16
AF = mybir.ActivationFunctionType
ALU = mybir.AluOpType

TAPS = [(dy, dx) for dy in range(3) for dx in range(3)]
CLIP = 20.0

N_CORES = 8
B_CORE = 8          # samples per core
NPAIR = B_CORE // 2

PATW = 4240         # per-partition conv0 patch buffer (elements)
PBASE = 65          # read-window base offset: window j in [PBASE, PBASE+4096)


def build_nc(debug=False):
    nc = bacc.Bacc(None, target_bir_lowering=False)

    inp = nc.dram_tensor("inputs", [B_CORE, 1, 64, 64], F32R, kind="ExternalInput")
    wc0 = nc.dram_tensor("w_conv0", [64, 1, 3, 3], F32, kind="ExternalInput")
    bc0 = nc.dram_tensor("b_conv0", [64], F32, kind="ExternalInput")
    wc1 = nc.dram_tensor("w_conv1", [64, 64, 3, 3], F32, kind="ExternalInput")
    bc1 = nc.dram_tensor("b_conv1", [64], F32, kind="ExternalInput")
    wen = nc.dram_tensor("w_enc", [1, 64, 3, 3], F32R, kind="ExternalInput")
    ben = nc.dram_tensor("b_enc", [1], F32, kind="ExternalInput")
    wc2 = nc.dram_tensor("w_conv2", [64, 1, 3, 3], F32, kind="ExternalInput")
    bc2 = nc.dram_tensor("b_conv2", [64], F32, kind="ExternalInput")
    wc3 = nc.dram_tensor("w_conv3", [64, 64, 3, 3], F32, kind="ExternalInput")
    bc3 = nc.dram_tensor("b_conv3", [64], F32, kind="ExternalInput")
    wc4 = nc.dram_tensor("w_conv4", [64, 64, 3, 3], F32, kind="ExternalInput")
    bc4 = nc.dram_tensor("b_conv4", [64], F32, kind="ExternalInput")
    wlx = nc.dram_tensor("w_lstm_x", [1024, 1024], F32R, kind="ExternalInput")
    bls = nc.dram_tensor("b_lstm", [1024], F32, kind="ExternalInput")
    wou = nc.dram_tensor("w_out", [1024, 256], F32R, kind="ExternalInput")
    bou = nc.dram_tensor("b_out", [256], F32R, kind="ExternalInput")
    out = nc.dram_tensor("out", [B_CORE, 64, 64, 64], F32, kind="ExternalOutput")

    dbg = {}
    if debug:
        for name, shape, dt in [
            ("dbg_h", [128, 2, 8], F32R),
            ("dbg_clip", [B_CORE, 16, 16], F32R),
            ("dbg_x", [B_CORE, 16, 16], F32),
            ("dbg_pat", [18, PATW], F32R),
            ("dbg_ct2", [128, 128], F32R),
            ("dbg_c1in", [128, 34, 34], F32R),
            ("dbg_pc2", [128, 684], F32R),
            ("dbg_ctc2", [128, 128], F32R),
        ]:
            dbg[name] = nc.dram_tensor(name, shape, dt, kind="ExternalOutput")

    with tile.TileContext(nc) as tc:
        with (
            tc.tile_pool(name="const", bufs=1) as const,
            tc.tile_pool(name="work", bufs=1) as work,
            tc.tile_pool(name="dbl", bufs=2) as dbl,
            tc.tile_pool(name="trip", bufs=3) as trip,
            tc.tile_pool(name="tri3", bufs=3) as tri3,
            tc.tile_pool(name="quad", bufs=4) as quad,
            tc.tile_pool(name="c3p", bufs=3) as c3p,
            tc.tile_pool(name="out2", bufs=2) as out2,
            tc.tile_pool(name="psmm", bufs=6, space="PSUM") as psmm,
            tc.tile_pool(name="psc3", bufs=2, space="PSUM") as psc3,
        ):
            dmaeng = [nc.sync, nc.gpsimd, nc.scalar]
            wrap_zero = bool(int(os.environ.get("KWRAP", "1")))

            # ---------------- conv0 patch buffer + pad memsets --------------
            # pairs 0-2 at base partitions 0/32/64; pair 3 (base partition
            # 96 is not a legal matmul operand base) lives at base 0 of a
            # second column range.
            # pair 3 lives at partitions 64-81 of a second tile so its patch
            # DMAs land on the ODD SDMA-engine set (partitions 64+) and
            # balance against pairs 0/1 on the even set.
            pat = const.tile([128, PATW], F32R, tag="pat")
            pat3 = const.tile([128, PATW], F32R, tag="pat3")
            patT = pat[:].tensor
            patO = pat[:].offset
            pat3T = pat3[:].tensor
            pat3O = pat3[:].offset + 64 * PATW
            # top/bottom row pads of the read window (shared by all taps);
            # emitted BEFORE the patch DMAs so interiors get overwritten.
            nc.vector.memset(pat[:, PBASE : PBASE + 65].bitcast(F32), 0.0)
            nc.vector.memset(pat[:, PBASE + 4031 : PBASE + 4096].bitcast(F32), 0.0)
            nc.vector.memset(pat3[:, PBASE : PBASE + 65].bitcast(F32), 0.0)
            nc.vector.memset(pat3[:, PBASE + 4031 : PBASE + 4096].bitcast(F32), 0.0)

            def pat_loc(p):
                if p < 3:
                    return patT, patO + 32 * p * PATW, pat
                return pat3T, pat3O, pat3

            # ---------------- conv0 patch DMAs: issued FIRST ----------------
            # Patch partition order r = 6dy + 3s + dx.  For a fixed dy all 6
            # partitions share the same trimmed dst window [d0, d0+4094)
            # (d0 = 66 - 64(dy-1)), and the (s, dx) tap shifts become LINEAR
            # source axes [[4096, 2], [1, 3]], so ONE 96KB HBM DMA fills the
            # whole dy-group: 3 DMAs per pair instead of 9.  (DMA AP shapes
            # need not match across sides; only the final contiguous dim
            # must.)  The few corner elements the common trim loses read as
            # the zero pad: ~4 real pixels per sample, negligible.
            pengs = [nc.sync, nc.scalar]
            if wrap_zero:
                zsrc = const.tile([128, 64], F32R, tag="zsrc")
                nc.vector.memset(zsrc[:].bitcast(F32), 0.0)

            # Partition order r = dy + 3s + 6dx: each dy-group is the
            # stride-3 set {dy, dy+3, .., dy+15} (spans ~4 SDMA engines
            # instead of 2), dx=0 is partitions 0-5 (aligned -> DVE memset
            # wrap-zero) and dx=2 is partitions 12-17 (one gpsimd zero-DMA).
            def patch_pair(p):
                pT, pO, _ = pat_loc(p)
                for dy in range(3):
                    d0 = 66 - 64 * (dy - 1)
                    (nc.sync if p < 2 else nc.gpsimd).dma_start(
                        out=bass.AP(
                            tensor=pT,
                            offset=pO + dy * PATW + d0,
                            ap=[[3 * PATW, 6], [1, 4094]],
                        ),
                        in_=bass.AP(
                            tensor=inp[:].tensor,
                            offset=2 * p * 4096,
                            ap=[[1, 3], [4096, 2], [1, 4094]],
                        ),
                    )

            def wrap_pair(p):
                # wrap-element zeroing, emitted AFTER all patch DMAs so the
                # completion waits never block further patch issues:
                #  dx=0: X=0 reads hit j = PBASE+64Y    -> zero [PBASE::64]
                #  dx=2: X=63 reads hit j = PBASE+63+64Y -> zero [PBASE+63::64]
                pT, pO, _ = pat_loc(p)
                nc.vector.memset(
                    bass.AP(
                        tensor=pT,
                        offset=pO + PBASE,
                        ap=[[PATW, 6], [64, 64]],
                    ).bitcast(F32),
                    0.0,
                )
                # partition base 12 is not DVE/gpsimd-writable (32-alignment
                # rule) -> zero-copy DMA for the dx=2 group.  MUST be HWDGE
                # (sync/scalar): SWDGE generates the 384 4B descriptors
                # serially on Q7 (~12us); HWDGE does it in RTL and spreads
                # them across all 16 SDMA slots (~1us).
                nc.sync.dma_start(
                    out=bass.AP(
                        tensor=pT,
                        offset=pO + 12 * PATW + PBASE + 63,
                        ap=[[PATW, 6], [64, 64]],
                    ),
                    in_=bass.AP(
                        tensor=zsrc[:].tensor,
                        offset=zsrc[:].offset,
                        ap=[[64, 6], [1, 64]],
                    ),
                )

            patch_pair(0)

            # stg2: conv2 padded staging rows (one partition per sample)
            stg2 = const.tile([8, 21, 19], F32R, tag="stg2")
            nc.vector.memset(stg2[:].bitcast(F32), 0.0)

            # ---------------- weight staging DMAs ---------------------------
            # s9x[c, 2q+s] (q = 3dx+dy) holds w_conv0[c%64, dy, dx] in the
            # (c<64) == (s==0) half, zero elsewhere; one PE transpose then
            # yields the conv0 lhsT block directly.  Staged via a t-order
            # load + 6 small strided gathers (all partition-step-1 APs).
            # s9t + gathers go on HWDGE (sync/scalar): as many-small-
            # descriptor DMAs they would serialize SWDGE's Q7 descriptor
            # generation for ~15us and gate ct2_c0 (and so conv0).
            s9t = const.tile([64, 9], F32, tag="s9t")
            nc.scalar.dma_start(out=s9t[:], in_=wc0[:].rearrange("a b c d -> a (b c d)"))
            s9x = const.tile([128, 18], F32, tag="s9x")
            nc.vector.memset(s9x[:], 0.0)
            for s in range(2):
                for dx in range(3):
                    nc.scalar.dma_start(
                        out=bass.AP(
                            tensor=s9x[:].tensor,
                            offset=s9x[:].offset + s * (64 * 18) + 3 * s + 6 * dx,
                            ap=[[18, 64], [1, 3]],
                        ),
                        in_=bass.AP(
                            tensor=s9t[:].tensor,
                            offset=s9t[:].offset + dx,
                            ap=[[9, 64], [3, 3]],
                        ),
                    )
            def load_wsrc(wdram):
                wsrc = dbl.tile([64, 576], F32, tag="wsrc")
                nc.gpsimd.dma_start(
                    out=wsrc[:], in_=wdram[:].rearrange("a b c d -> a (b c d)")
                )
                return wsrc

            # ---------------- conv biases (bt0/bt1 needed by the encoder
            # evictions; the rest load after the patch DMAs) ---------------
            def bias128(dram_b, tag, eng):
                bt = const.tile([128, 1], F32, tag=tag)
                eng.dma_start(out=bt[0:64, :], in_=dram_b[:].unsqueeze(1))
                eng.dma_start(out=bt[64:128, :], in_=dram_b[:].unsqueeze(1))
                return bt

            patch_pair(1)
            patch_pair(2)
            patch_pair(3)
            bt0 = bias128(bc0, "bt0", nc.gpsimd)
            bt1 = bias128(bc1, "bt1", nc.gpsimd)
            bt0s = const.tile([128, 1], F32, tag="bt0s")
            nc.vector.tensor_scalar_mul(bt0s[:], bt0[:], 0.25)
            bt1s = const.tile([128, 1], F32, tag="bt1s")
            nc.vector.tensor_scalar_mul(bt1s[:], bt1[:], 0.25)
            # pair-0's wrap zero issues right after the s9x gathers so
            # conv0 can start at ~16us; later pairs' zeros interleave with
            # the remaining weight loads.
            if wrap_zero:
                wrap_pair(0)
                wrap_pair(1)
            wsrc_c1 = load_wsrc(wc1)
            if wrap_zero:
                wrap_pair(2)
                wrap_pair(3)
            # ---------------- identity (for PE transposes) ------------------
            ident = const.tile([128, 128], F32, tag="ident")
            make_identity(nc, ident)

            # ---------------- PE warmup -------------------------------------
            # The HAM clock gate holds the PE at 1.2GHz until it has been
            # busy for a full 3.4us window.  While the patch DMAs stage,
            # keep the PE streaming dummy matmuls so the encoder starts at
            # 2.4GHz instead of half clock.
            wgarb = const.tile([128, 512], F32R, tag="wgarb")
            nc.vector.memset(wgarb[:].bitcast(F32), 0.0)

            def warm(n):
                # N=512 dummies: 213ns of guaranteed-dense PE busy each, so
                # a burst of ~16 always covers the 3.4us HAM re-warm window
                for _ in range(n):
                    pwu = psmm.tile([64, 512], F32, tag="mm")
                    nc.tensor.matmul(
                        pwu[:], wgarb[:, 0:64], wgarb[:], start=True, stop=True
                    )

            warm(62)

            # ---------------- 1ch conv weights ------------------------------
            # staged t-order [9, 64] via PE transpose, then scattered to the
            # block-diagonal replicated lhsT tiles with one remap DMA each.
            ct2_c0 = const.tile([128, 128], F32R, tag="ct2_c0")
            nc.vector.memset(ct2_c0[:].bitcast(F32), 0.0)
            ct2_c2 = const.tile([128, 128], F32R, tag="ct2_c2")
            nc.vector.memset(ct2_c2[:].bitcast(F32), 0.0)

            p9 = psmm.tile([18, 128], F32, tag="mm")
            nc.tensor.transpose(p9[:], s9x[:], ident[0:128, 0:128])
            nc.scalar.activation(
                ct2_c0[0:18, :], p9[:], AF.Copy, bias=0.0, scale=1.0
            )
            for p in (1, 2):
                nc.sync.dma_start(
                    out=ct2_c0[32 * p : 32 * p + 18, :],
                    in_=ct2_c0[0:18, :],
                )


            # ---------------- 64ch conv weights -> block-diag lhsT ----------
            wtap = {}

            def build_wtap(name, wsrc):
                wt = const.tile([128, 9, 128], B16, tag=f"wtap_{name}")
                nc.vector.memset(wt[:], 0.0)
                for t in range(9):
                    pw = psmm.tile([64, 64], F32, tag="mm")
                    nc.tensor.transpose(pw[:], wsrc[:, t::9], ident[0:64, 0:64])
                    nc.scalar.activation(
                        wt[0:64, t, 0:64], pw[:], AF.Copy, bias=0.0, scale=1.0
                    )
                nc.sync.dma_start(out=wt[64:128, :, 64:128], in_=wt[0:64, :, 0:64])
                wtap[name] = wt

            build_wtap("c1", wsrc_c1)

            # enc conv weights (64ci -> 1co): one gather DMA, no scaling.
            encT = const.tile([128, 9, 2], F32R, tag="encT")
            nc.vector.memset(encT[:].bitcast(F32), 0.0)
            # out (c+64s)*18 + 2t + s  <-  wen flat c*9 + t  (one DMA per s)
            for s in range(2):
                nc.scalar.dma_start(
                    out=bass.AP(
                        tensor=encT[:].tensor,
                        offset=encT[:].offset + s * (64 * 18 + 1),
                        ap=[[18, 64], [2, 9]],
                    ),
                    in_=bass.AP(
                        tensor=wen[:].tensor,
                        offset=0,
                        ap=[[9, 64], [1, 9]],
                    ),
                )

            bte = const.tile([2, 1], F32, tag="bte")
            nc.sync.dma_start(
                out=bte[:],
                in_=bass.AP(tensor=ben[:].tensor, offset=0, ap=[[0, 2], [1, 1]]),
            )

            xstage = const.tile([8, 16, 16], F32, tag="xstage")

            # ================ encoder: interleaved over 4 sample pairs ======
            c1in_l = [None] * NPAIR
            ein_l = [None] * NPAIR

            def conv0_pair(p):
                c1in = tri3.tile([128, 34, 34], B16, tag="c1in")
                nc.gpsimd.memset(c1in[:, 0:1, :], 0.0)
                nc.gpsimd.memset(c1in[:, 33:34, :], 0.0)
                nc.gpsimd.memset(c1in[:, 1:33, 0:1], 0.0)
                nc.gpsimd.memset(c1in[:, 1:33, 33:34], 0.0)
                base = 32 * p if p < 3 else 64
                _, _, ptile = pat_loc(p)
                for n in range(8):
                    ps = psmm.tile([128, 4, 2, 32, 2], F32, tag="mm")
                    nc.tensor.matmul(
                        ps[:].rearrange("p a b c d -> p (a b c d)"),
                        ct2_c0[base : base + 18, :],
                        ptile[base : base + 18, PBASE + 512 * n : PBASE + 512 * (n + 1)],
                        start=True,
                        stop=True,
                    )
                    ct0 = trip.tile([128, 4, 2, 32, 2], F32, tag="ct0")
                    nc.scalar.activation(ct0[:], ps[:], AF.Relu, bias=bt0s, scale=0.25)
                    tcol = tri3.tile([128, 4, 2, 32], F32, tag="tcol")
                    nc.vector.tensor_add(
                        tcol[:], ct0[:, :, :, :, 0], ct0[:, :, :, :, 1]
                    )
                    nc.vector.tensor_add(
                        c1in[:, 1 + 4 * n : 5 + 4 * n, 1:33],
                        tcol[:, :, 0, :],
                        tcol[:, :, 1, :],
                    )
                c1in_l[p] = c1in

            def conv1_pair(p):
                c1in = c1in_l[p]
                e_in = quad.tile([128, 18, 18], F32R, tag="e_in")
                nc.gpsimd.memset(e_in[:, 0:1, :].bitcast(F32), 0.0)
                nc.gpsimd.memset(e_in[:, 17:18, :].bitcast(F32), 0.0)
                nc.gpsimd.memset(e_in[:, 1:17, 0:1].bitcast(F32), 0.0)
                nc.gpsimd.memset(e_in[:, 1:17, 17:18].bitcast(F32), 0.0)
                for n in range(2):
                    ps = psmm.tile([128, 8, 2, 16, 2], F32, tag="mm")
                    for t, (dy, dx) in enumerate(TAPS):
                        nc.tensor.matmul(
                            ps[:],
                            wtap["c1"][:, t, :],
                            c1in[:, n * 16 + dy : n * 16 + dy + 16, dx : dx + 32],
                            start=(t == 0),
                            stop=(t == 8),
                        )
                    ct1 = trip.tile([128, 8, 2, 16, 2], F32, tag="ct1")
                    nc.scalar.activation(ct1[:], ps[:], AF.Relu, bias=bt1s, scale=0.25)
                    tc1 = tri3.tile([128, 8, 2, 16], F32, tag="tc1")
                    nc.vector.tensor_add(
                        tc1[:], ct1[:, :, :, :, 0], ct1[:, :, :, :, 1]
                    )
                    nc.vector.tensor_add(
                        e_in[:, 1 + 8 * n : 9 + 8 * n, 1:17],
                        tc1[:, :, 0, :],
                        tc1[:, :, 1, :],
                    )
                ein_l[p] = e_in

            def enc_pair(p):
                e_in = ein_l[p]
                pe = psmm.tile([2, 16, 16], F32, tag="mm")
                for t, (dy, dx) in enumerate(TAPS):
                    nc.tensor.matmul(
                        pe[:],
                        encT[:, t, :],
                        e_in[:, dy : dy + 16, dx : dx + 16],
                        start=(t == 0),
                        stop=(t == 8),
                    )
                estage = dbl.tile([2, 16, 16], F32, tag="estage")
                nc.scalar.activation(estage[:], pe[:], AF.Relu, bias=bte)
                nc.scalar.dma_start(out=xstage[2 * p : 2 * p + 2, :, :], in_=estage[:])

            conv0_pair(0)
            if debug:
                nc.sync.dma_start(out=dbg["dbg_pat"][:], in_=pat[0:18, :])
                nc.sync.dma_start(out=dbg["dbg_ct2"][:], in_=ct2_c0[:])
                nc.sync.dma_start(out=dbg["dbg_c1in"][:], in_=c1in_l[0][:])
            conv0_pair(1)
            warm(10)
            conv1_pair(0)
            warm(1)
            conv0_pair(2)
            enc_pair(0)
            conv1_pair(1)
            warm(1)
            conv0_pair(3)
            enc_pair(1)
            conv1_pair(2)
            warm(1)
            enc_pair(2)
            conv1_pair(3)
            warm(1)
            enc_pair(3)

            # ---------------- deferred weight staging (decoder + NTM): -----
            # emitted after the encoder so these HBM loads queue BEHIND the
            # conv0 patch DMAs and don't stall the pipeline start; they
            # drain during the encoder compute.
            # s9y[c, 9s+t] = w_conv2[c%64, t] in the matching half (t-order).
            s9y = const.tile([128, 18], F32, tag="s9y")
            nc.vector.memset(s9y[:], 0.0)
            for s in range(2):
                nc.scalar.dma_start(
                    out=bass.AP(
                        tensor=s9y[:].tensor,
                        offset=s9y[:].offset + s * (64 * 18 + 9),
                        ap=[[18, 64], [1, 9]],
                    ),
                    in_=wc2[:].rearrange("a b c d -> a (b c d)"),
                )
            p9b = psmm.tile([18, 128], F32, tag="mm")
            nc.tensor.transpose(p9b[:], s9y[:], ident[0:128, 0:128])
            nc.scalar.activation(
                ct2_c2[0:18, :], p9b[:], AF.Copy, bias=0.0, scale=1.0
            )
            for p in (1, 2):
                nc.scalar.dma_start(
                    out=ct2_c2[32 * p : 32 * p + 18, :],
                    in_=ct2_c2[0:18, :],
                )
            bt2 = bias128(bc2, "bt2", nc.scalar)
            bt3 = bias128(bc3, "bt3", nc.sync)
            bt4 = bias128(bc4, "bt4", nc.scalar)

            # NTM weights: w_lstm_x rows 0:256, gate cols i/g/o
            wx = const.tile([128, 2, 768], F32R, tag="wx")
            for kt in range(2):
                nc.gpsimd.dma_start(
                    out=wx[:, kt, 0:256],
                    in_=wlx[kt * 128 : (kt + 1) * 128, 0:256],
                )
                nc.gpsimd.dma_start(
                    out=wx[:, kt, 256:768],
                    in_=wlx[kt * 128 : (kt + 1) * 128, 512:1024],
                )
            bigo = const.tile([128, 6], F32, tag="bigo")
            # cols (2j+h2): j in (i,g,o) -> b_lstm[0:256] and b_lstm[512:1024]
            nc.sync.dma_start(
                out=bass.AP(tensor=bigo[:].tensor, offset=bigo[:].offset,
                            ap=[[6, 128], [1, 2]]),
                in_=bass.AP(tensor=bls[:].tensor, offset=0,
                            ap=[[1, 128], [128, 2]]),
            )
            nc.sync.dma_start(
                out=bass.AP(tensor=bigo[:].tensor, offset=bigo[:].offset + 2,
                            ap=[[6, 128], [1, 4]]),
                in_=bass.AP(tensor=bls[:].tensor, offset=512,
                            ap=[[1, 128], [128, 4]]),
            )
            # w_out rows 0:256 (h part) + bias row
            wo = const.tile([128, 2, 256], F32R, tag="wo")
            nc.gpsimd.dma_start(out=wo[:, 0, :], in_=wou[0:128, :])
            nc.gpsimd.dma_start(out=wo[:, 1, :], in_=wou[128:256, :])
            rhs_b = const.tile([1, 256], F32R, tag="rhs_b")
            nc.scalar.dma_start(out=rhs_b[:], in_=bou[:].unsqueeze(0))
            ones1 = const.tile([1, 8], F32R, tag="ones1")
            nc.vector.memset(ones1[:].bitcast(F32), 1.0)

            # deferred weight prep: FOLDED decoder kernels (fills the PE
            # bubble while the NTM chain runs).  conv3/conv4 consume a 2x
            # nearest-neighbor-upsampled input, so conv(up2(X)) collapses
            # into 4 output-phase convolutions with 2x2 kernels over the
            # un-upsampled X: phase (a,b) kernel (ey,ex) = sum of W[dy,dx]
            # over dy in GRP[a][ey], dx in GRP[b][ex].  The tap sums are
            # accumulated directly in PSUM by the transposes.
            GRP = (((0,), (1, 2)), ((0, 1), (2,)))
            wfold = {}

            def build_wfold(name, wsrc):
                wf = const.tile([128, 16, 128], B16, tag=f"wfold_{name}")
                nc.vector.memset(wf[:], 0.0)
                for a in range(2):
                    for b in range(2):
                        for ey in range(2):
                            for ex in range(2):
                                k = 8 * a + 4 * b + 2 * ey + ex
                                taps = [
                                    3 * dy + dx
                                    for dy in GRP[a][ey]
                                    for dx in GRP[b][ex]
                                ]
                                pw = psmm.tile([64, 64], F32, tag="mm")
                                for i, t in enumerate(taps):
                                    nc.tensor.matmul(
                                        pw[:],
                                        wsrc[:, t::9],
                                        ident[0:64, 0:64],
                                        is_transpose=True,
                                        start=(i == 0),
                                        stop=(i == len(taps) - 1),
                                    )
                                nc.scalar.activation(
                                    wf[0:64, k, 0:64], pw[:], AF.Copy,
                                    bias=0.0, scale=1.0,
                                )
                nc.sync.dma_start(out=wf[64:128, :, 64:128], in_=wf[0:64, :, 0:64])
                wfold[name] = wf

            wsrc_c3 = load_wsrc(wc3)
            build_wfold("c3", wsrc_c3)
            wsrc_c4 = load_wsrc(wc4)
            build_wfold("c4", wsrc_c4)
            # bridge the gap while the enc evictions land in xstage
            warm(5)

            # ================ NTM step (all 8 samples at once) ==============
            if debug:
                nc.sync.dma_start(out=dbg["dbg_x"][:], in_=xstage[:])
            # x^T k-tiles via PE transpose
            xT = work.tile([128, 2, 8], F32R, tag="xT")
            for kt in range(2):
                pxt = psmm.tile([128, 8], F32, tag="mm")
                nc.tensor.transpose(
                    pxt[:],
                    xstage[:].rearrange("p a b -> p (a b)")[:, kt * 128 : kt * 128 + 128],
                    ident[0:8, 0:8],
                )
                nc.scalar.activation(xT[:, kt, :], pxt[:], AF.Copy, bias=0.0, scale=1.0)
            # z = x @ Wx + b for gates i, g, o; h = sig(o) * tanh(sig(i)*tanh(g))
            zps = psmm.tile([128, 6, 8], F32, tag="mm")
            for j in range(3):
                for h2 in range(2):
                    for kt in range(2):
                        nc.tensor.matmul(
                            zps[:, 2 * j + h2, :],
                            wx[:, kt, j * 256 + h2 * 128 : j * 256 + h2 * 128 + 128],
                            xT[:, kt, :],
                            start=(kt == 0),
                            stop=(kt == 1),
                        )
            zb = work.tile([128, 6, 8], F32, tag="zb")
            bigo_b = bass.AP(
                tensor=bigo[:].tensor, offset=bigo[:].offset,
                ap=[list(d) for d in bigo[:].ap] + [[0, 8]],
            )
            nc.vector.tensor_tensor(zb[:], zps[:], bigo_b, op=ALU.add)
            si = work.tile([128, 2, 8], F32, tag="gate0")
            nc.scalar.activation(si[:], zb[:, 0:2, :], AF.Sigmoid, bias=0.0)
            tg = work.tile([128, 2, 8], F32, tag="gate1")
            nc.scalar.activation(tg[:], zb[:, 2:4, :], AF.Tanh, bias=0.0)
            so = work.tile([128, 2, 8], F32, tag="gate2")
            nc.scalar.activation(so[:], zb[:, 4:6, :], AF.Sigmoid, bias=0.0)
            ctile = work.tile([128, 2, 8], F32, tag="ctile")
            nc.vector.tensor_mul(ctile[:], si[:], tg[:])
            tct = work.tile([128, 2, 8], F32, tag="tct")
            nc.scalar.activation(tct[:], ctile[:], AF.Tanh, bias=0.0)
            h = work.tile([128, 2, 8], F32R, tag="h")
            nc.vector.tensor_mul(h[:], so[:], tct[:])
            if debug:
                nc.sync.dma_start(out=dbg["dbg_h"][:], in_=h[:])
            # out = clip(h @ w_out[:256] + b_out)  (reads contribution dropped)
            pout = psmm.tile([8, 16, 16], F32, tag="mm")
            for kt in range(2):
                nc.tensor.matmul(
                    pout[:].rearrange("p a b -> p (a b)"),
                    h[:, kt, :],
                    wo[:, kt, :],
                    start=(kt == 0),
                    stop=False,
                )
            nc.tensor.matmul(
                pout[:].rearrange("p a b -> p (a b)"),
                ones1[:],
                rhs_b[:],
                start=False,
                stop=True,
            )
            nc.vector.tensor_scalar(
                stg2[:, 1:17, 1:17], pout[:], -CLIP, CLIP, ALU.max, ALU.min
            )
            # keep the PE clock warm while the NTM result fans out through
            # stg2 -> pc2 staging DMAs: the HAM gate needs a DENSE ~3.4us
            # busy window to hold/raise 2.4GHz, and a cold decoder start
            # costs ~15us.
            warm(18)
            if debug:
                nc.sync.dma_start(out=dbg["dbg_clip"][:], in_=stg2[:, 1:17, 1:17])

            # ================ decoder: stage-major over 4 pairs =============
            # conv2 patches for all pairs in one merged tile + one DMA.
            # partition r = 32p + 9s + 3dy + dx via the overlapping stride-1
            # dx trick (reads stg2 shifted by 0/1/2 columns).
            pc2 = const.tile([128, 684], F32R, tag="pc2")
            for p in range(NPAIR):
                base = 32 * p if p < 3 else 0
                c0 = 0 if p < 3 else 342
                for s in range(2):
                    for dy in range(3):
                        eng = dmaeng[(2 * p + s + dy) % 3]
                        eng.dma_start(
                            out=bass.AP(
                                tensor=pc2[:].tensor,
                                offset=pc2[:].offset
                                + (base + 9 * s + 3 * dy) * 684 + c0,
                                ap=[[684, 3], [1, 341]],
                            ),
                            in_=bass.AP(
                                tensor=stg2[:].tensor,
                                offset=stg2[:].offset + (2 * p + s) * 399 + dy * 19,
                                ap=[[399, 1], [1, 3], [1, 341]],
                            ),
                        )

            # --- conv2 all pairs -> padded stage S2 (reuses the quad pool
            # slots freed by the enc pairs; 1 eviction per pair instead of
            # the old 4 upsample writes)
            c2s_l = []
            for p in range(NPAIR):
                base = 32 * p if p < 3 else 0
                c0 = 0 if p < 3 else 342
                ps2 = psmm.tile([128, 16, 16], F32, tag="mm")
                nc.tensor.matmul(
                    ps2[:],
                    ct2_c2[base : base + 18, :],
                    pc2[base : base + 18, c0 : c0 + 342]
                    .rearrange("p (a b) -> p a b", a=18)[:, 0:16, 0:16],
                    start=True,
                    stop=True,
                )
                S2 = quad.tile([128, 18, 18], B16, tag="c2s")
                nc.gpsimd.memset(S2[:, 0:1, :], 0.0)
                nc.gpsimd.memset(S2[:, 17:18, :], 0.0)
                nc.gpsimd.memset(S2[:, 1:17, 0:1], 0.0)
                nc.gpsimd.memset(S2[:, 1:17, 17:18], 0.0)
                nc.scalar.activation(S2[:, 1:17, 1:17], ps2[:], AF.Relu, bias=bt2)
                c2s_l.append(S2)
            if debug:
                nc.sync.dma_start(out=dbg["dbg_pc2"][:], in_=pc2[:])
                nc.sync.dma_start(out=dbg["dbg_ctc2"][:], in_=ct2_c2[:])

            # --- conv3 folded: 4 output-phase 2x2 convs over S2 -> S3
            # (34x34 padded, UN-upsampled conv3 output)
            c3s_l = [None] * NPAIR

            def conv3_pair(p):
                S2 = c2s_l[p]
                S3 = c3p.tile([128, 34, 34], B16, tag="c3s")
                nc.gpsimd.memset(S3[:, 0:1, :], 0.0)
                nc.gpsimd.memset(S3[:, 33:34, :], 0.0)
                nc.gpsimd.memset(S3[:, 1:33, 0:1], 0.0)
                nc.gpsimd.memset(S3[:, 1:33, 33:34], 0.0)
                S3v = S3[:].rearrange(
                    "p (ri ra) (ci cb) -> p ri ra ci cb", ra=2, cb=2
                )
                for a in range(2):
                    for b in range(2):
                        ps = psc3.tile([128, 16, 16], F32, tag="mm3")
                        i = 0
                        for ey in range(2):
                            for ex in range(2):
                                k = 8 * a + 4 * b + 2 * ey + ex
                                nc.tensor.matmul(
                                    ps[:],
                                    wfold["c3"][:, k, :],
                                    S2[:, a + ey : a + ey + 16,
                                       b + ex : b + ex + 16],
                                    start=(i == 0),
                                    stop=(i == 3),
                                )
                                i += 1
                        # out row 1+2r+a, col 1+2c+b in S3
                        rs = slice(0, 16) if a == 0 else slice(1, 17)
                        cs = slice(0, 16) if b == 0 else slice(1, 17)
                        dst = S3v[:, rs, 1 - a, cs, 1 - b]
                        if (a + b) % 2 == 0:
                            nc.scalar.activation(dst, ps[:], AF.Relu, bias=bt3)
                        else:
                            nc.vector.tensor_scalar(
                                dst, ps[:], bt3[:], 0.0, ALU.add, ALU.max
                            )
                c3s_l[p] = S3

            # --- conv4 folded: phases over S3; each 32-row block stores as
            # one 1MB DMA as soon as its 4 phases are evicted.
            store_rot = [nc.gpsimd, nc.sync]

            def conv4_pair(p):
                S3 = c3s_l[p]
                c4out = out2.tile([128, 64, 64], F32, tag="c4out")
                c4v = c4out[:].rearrange(
                    "p (ri ra) (ci cb) -> p ri ra ci cb", ra=2, cb=2
                )
                for h in range(2):
                    for a in range(2):
                        for b in range(2):
                            ps = psmm.tile([128, 16, 32], F32, tag="mm")
                            i = 0
                            for ey in range(2):
                                for ex in range(2):
                                    k = 8 * a + 4 * b + 2 * ey + ex
                                    nc.tensor.matmul(
                                        ps[:],
                                        wfold["c4"][:, k, :],
                                        S3[:, 16 * h + a + ey : 16 * h + a + ey + 16,
                                           b + ex : b + ex + 32],
                                        start=(i == 0),
                                        stop=(i == 3),
                                    )
                                    i += 1
                            dst = c4v[:, 16 * h : 16 * h + 16, a, 0:32, b]
                            if (a + b) % 2 == 0:
                                nc.scalar.activation(dst, ps[:], AF.Relu, bias=bt4)
                            else:
                                nc.vector.tensor_scalar(
                                    dst, ps[:], bt4[:], 0.0, ALU.add, ALU.max
                                )
                    store_rot[(2 * p + h) % 2].dma_start(
                        out=bass.AP(
                            tensor=out[:].tensor,
                            offset=2 * p * 262144 + 2048 * h,
                            ap=[[4096, 128], [1, 2048]],
                        ),
                        in_=bass.AP(
                            tensor=c4out[:].tensor,
                            offset=c4out[:].offset + 2048 * h,
                            ap=[[4096, 128], [1, 2048]],
                        ),
                    )

            conv3_pair(0)
            conv3_pair(1)
            conv4_pair(0)
            conv3_pair(2)
            conv4_pair(1)
            conv3_pair(3)
            conv4_pair(2)
            conv4_pair(3)

    nc.compile()
    return nc


_NC_CACHE = {}
LAST_RESULT = None

WEIGHT_NAMES = [
    "w_conv0", "b_conv0", "w_conv1", "b_conv1", "w_enc", "b_enc",
    "w_conv2", "b_conv2", "w_conv3", "b_conv3", "w_conv4", "b_conv4",
    "w_lstm_x", "b_lstm", "w_out", "b_out",
]


def kernel(**inputs):
    global LAST_RESULT
    from concourse.bass_utils import run_bass_kernel_spmd

    debug = bool(int(os.environ.get("KDEBUG", "0")))
    key = ("nc", debug)
    if key not in _NC_CACHE:
        _NC_CACHE[key] = build_nc(debug=debug)
    nc = _NC_CACHE[key]

    xs = np.ascontiguousarray(np.asarray(inputs["inputs"], dtype=np.float32))
    weights = {
        k: np.ascontiguousarray(np.asarray(inputs[k], dtype=np.float32))
        for k in WEIGHT_NAMES
    }
    in_maps = []
    for c in range(N_CORES):
        m = dict(weights)
        m["inputs"] = xs[c * B_CORE : (c + 1) * B_CORE]
        in_maps.append(m)

    res = run_bass_kernel_spmd(nc, in_maps, core_ids=list(range(N_CORES)))
    LAST_RESULT = res
    return np.concatenate([r["out"] for r in res.results], axis=0)


if __name__ == "__main__":
    nc = build_nc()
    print("built ok")



# revision 83
# speedup vs baseline: 1.0293x; 1.0160x over previous
"""Trainium2 Bass kernel for nn_Encoder_Decoder_Wrapper (conv encoder -> NTM step -> conv decoder).

Sharding: pure data parallel, batch 64 -> 8 cores x 8 samples; weights
replicated.  Per core, samples run in 4 pairs of 2 so every 64-channel conv
is a K=128/M=128 block-diagonal matmul (2 samples packed in contraction and
output partitions).

conv0 patch staging: 9 tap-shifted image copies per pair, partition order
r = dy + 3s + 6dx, so one 96KB HBM DMA per (pair, dy) fills a stride-3
partition set with the (s, dx) tap shifts expressed as linear source axes
(12 DMAs total).  dx=0 row-wrap elements are killed by an aligned DVE
strided memset, dx=2 by an HWDGE zero-copy DMA (SWDGE would generate its
384 4-byte descriptors serially on Q7, ~12us).  Pair 3 sits at partitions
64-81 of a second tile to use the odd SDMA-engine set.  conv0 stays fp32r
(its rhs streams straight from the f32 input).

NTM step reduced via its constant initial state: reads0 = h0 = c0 = 0, so
only out = clip(h @ w_out[:256] + b_out) with h = sig(o)*tanh(sig(i)*tanh(g));
the dropped ~1e-6 read vectors change the output by ~2e-4 relative.

Decoder: conv3/conv4 consume 2x nearest-neighbor-upsampled inputs, so
conv(up2(X)) is FOLDED into 4 output-phase convolutions with 2x2 kernels
over the un-upsampled X (phase (a,b) kernel (ey,ex) = sum of W[dy,dx] over
dy in GRP[a][ey], dx in GRP[b][ex], summed in PSUM by the weight-prep
transposes): 2.25x fewer matmul columns and no upsample staging.  Each
32-row conv4 block stores as one 1MB single-partition-axis DMA as soon as
its 4 phases evict, fully overlapping the 8MB writeback.

conv1 and the decoder run in bfloat16 (weights + staged activations, fp32
PSUM accumulation): fp32r matmuls execute two-pass (fp32_mode=LOW_HIGH)
with a ~190ns fused 4-byte weight load that dominates short tiles; bf16
streams single-pass with half-size weight loads and doubles DVE eviction
throughput.  Total rel err ~6e-3 vs the 2e-2 budget.

Dummy N=512 matmul bursts ("warm") keep the PE HAM clock gate at 2.4GHz
through the patch-staging window and the NTM serial section: the gate
falls to 1.2GHz after ~3.4us idle and needs ~3.4us of dense busy to
recover, so each cold dip costs ~10us.  Bursts must be emitted BEFORE any
instruction that blocks on a slow load (engine queues are FIFO).
"""

import os
import sys

sys.path.insert(0, "/opt/trn_rl_repo")
os.environ.setdefault("MYCRO_LOCAL_CACHE", "1")

import numpy as np

import concourse.bass as bass
import concourse.bacc as bacc
import concourse.mybir as mybir
import concourse.tile as tile
from concourse.masks import make_identity

F32 = mybir.dt.float32
F32R = mybir.dt.float32r
B16 = mybir.dt.bfloat16
AF = mybir.ActivationFunctionType
ALU = mybir.AluOpType

TAPS = [(dy, dx) for dy in range(3) for dx in range(3)]
CLIP = 20.0

N_CORES = 8
B_CORE = 8          # samples per core
NPAIR = B_CORE // 2

PATW = 4240         # per-partition conv0 patch buffer (elements)
PBASE = 65          # read-window base offset: window j in [PBASE, PBASE+4096)


def build_nc(debug=False):
    nc = bacc.Bacc(None, target_bir_lowering=False)

    inp = nc.dram_tensor("inputs", [B_CORE, 1, 64, 64], F32R, kind="ExternalInput")
    wc0 = nc.dram_tensor("w_conv0", [64, 1, 3, 3], F32, kind="ExternalInput")
    bc0 = nc.dram_tensor("b_conv0", [64], F32, kind="ExternalInput")
    wc1 = nc.dram_tensor("w_conv1", [64, 64, 3, 3], F32, kind="ExternalInput")
    bc1 = nc.dram_tensor("b_conv1", [64], F32, kind="ExternalInput")
    wen = nc.dram_tensor("w_enc", [1, 64, 3, 3], F32R, kind="ExternalInput")
    ben = nc.dram_tensor("b_enc", [1], F32, kind="ExternalInput")
    wc2 = nc.dram_tensor("w_conv2", [64, 1, 3, 3], F32, kind="ExternalInput")
    bc2 = nc.dram_tensor("b_conv2", [64], F32, kind="ExternalInput")
    wc3 = nc.dram_tensor("w_conv3", [64, 64, 3, 3], F32, kind="ExternalInput")
    bc3 = nc.dram_tensor("b_conv3", [64], F32, kind="ExternalInput")
    wc4 = nc.dram_tensor("w_conv4", [64, 64, 3, 3], F32, kind="ExternalInput")
    bc4 = nc.dram_tensor("b_conv4", [64], F32, kind="ExternalInput")
    wlx = nc.dram_tensor("w_lstm_x", [1024, 1024], F32R, kind="ExternalInput")
    bls = nc.dram_tensor("b_lstm", [1024], F32, kind="ExternalInput")
    wou = nc.dram_tensor("w_out", [1024, 256], F32R, kind="ExternalInput")
    bou = nc.dram_tensor("b_out", [256], F32R, kind="ExternalInput")
    out = nc.dram_tensor("out", [B_CORE, 64, 64, 64], F32, kind="ExternalOutput")

    dbg = {}
    if debug:
        for name, shape, dt in [
            ("dbg_h", [128, 2, 8], F32R),
            ("dbg_clip", [B_CORE, 16, 16], F32R),
            ("dbg_x", [B_CORE, 16, 16], F32),
            ("dbg_pat", [18, PATW], F32R),
            ("dbg_ct2", [128, 128], F32R),
            ("dbg_c1in", [128, 34, 34], F32R),
            ("dbg_pc2", [128, 684], F32R),
            ("dbg_ctc2", [128, 128], F32R),
        ]:
            dbg[name] = nc.dram_tensor(name, shape, dt, kind="ExternalOutput")

    with tile.TileContext(nc) as tc:
        with (
            tc.tile_pool(name="const", bufs=1) as const,
            tc.tile_pool(name="work", bufs=1) as work,
            tc.tile_pool(name="dbl", bufs=2) as dbl,
            tc.tile_pool(name="trip", bufs=3) as trip,
            tc.tile_pool(name="tri3", bufs=3) as tri3,
            tc.tile_pool(name="quad", bufs=4) as quad,
            tc.tile_pool(name="c3p", bufs=3) as c3p,
            tc.tile_pool(name="out2", bufs=2) as out2,
            tc.tile_pool(name="psmm", bufs=6, space="PSUM") as psmm,
            tc.tile_pool(name="psc3", bufs=2, space="PSUM") as psc3,
        ):
            dmaeng = [nc.sync, nc.gpsimd, nc.scalar]
            wrap_zero = bool(int(os.environ.get("KWRAP", "1")))

            # ---------------- conv0 patch buffer + pad memsets --------------
            # pairs 0-2 at base partitions 0/32/64; pair 3 (base partition
            # 96 is not a legal matmul operand base) lives at base 0 of a
            # second column range.
            # pair 3 lives at partitions 64-81 of a second tile so its patch
            # DMAs land on the ODD SDMA-engine set (partitions 64+) and
            # balance against pairs 0/1 on the even set.
            pat = const.tile([128, PATW], F32R, tag="pat")
            pat3 = const.tile([128, PATW], F32R, tag="pat3")
            patT = pat[:].tensor
            patO = pat[:].offset
            pat3T = pat3[:].tensor
            pat3O = pat3[:].offset + 64 * PATW
            # top/bottom row pads of the read window (shared by all taps);
            # emitted BEFORE the patch DMAs so interiors get overwritten.
            nc.vector.memset(pat[:, PBASE : PBASE + 65].bitcast(F32), 0.0)
            nc.vector.memset(pat[:, PBASE + 4031 : PBASE + 4096].bitcast(F32), 0.0)
            nc.vector.memset(pat3[:, PBASE : PBASE + 65].bitcast(F32), 0.0)
            nc.vector.memset(pat3[:, PBASE + 4031 : PBASE + 4096].bitcast(F32), 0.0)

            def pat_loc(p):
                if p < 3:
                    return patT, patO + 32 * p * PATW, pat
                return pat3T, pat3O, pat3

            # ---------------- conv0 patch DMAs: issued FIRST ----------------
            # Patch partition order r = 6dy + 3s + dx.  For a fixed dy all 6
            # partitions share the same trimmed dst window [d0, d0+4094)
            # (d0 = 66 - 64(dy-1)), and the (s, dx) tap shifts become LINEAR
            # source axes [[4096, 2], [1, 3]], so ONE 96KB HBM DMA fills the
            # whole dy-group: 3 DMAs per pair instead of 9.  (DMA AP shapes
            # need not match across sides; only the final contiguous dim
            # must.)  The few corner elements the common trim loses read as
            # the zero pad: ~4 real pixels per sample, negligible.
            pengs = [nc.sync, nc.scalar]
            if wrap_zero:
                zsrc = const.tile([128, 64], F32R, tag="zsrc")
                nc.vector.memset(zsrc[:].bitcast(F32), 0.0)

            # Partition order r = dy + 3s + 6dx: each dy-group is the
            # stride-3 set {dy, dy+3, .., dy+15} (spans ~4 SDMA engines
            # instead of 2), dx=0 is partitions 0-5 (aligned -> DVE memset
            # wrap-zero) and dx=2 is partitions 12-17 (one gpsimd zero-DMA).
            def patch_pair(p):
                pT, pO, _ = pat_loc(p)
                for dy in range(3):
                    d0 = 66 - 64 * (dy - 1)
                    (nc.sync if p < 2 else nc.gpsimd).dma_start(
                        out=bass.AP(
                            tensor=pT,
                            offset=pO + dy * PATW + d0,
                            ap=[[3 * PATW, 6], [1, 4094]],
                        ),
                        in_=bass.AP(
                            tensor=inp[:].tensor,
                            offset=2 * p * 4096,
                            ap=[[1, 3], [4096, 2], [1, 4094]],
                        ),
                    )

            def wrap_pair(p):
                # wrap-element zeroing, emitted AFTER all patch DMAs so the
                # completion waits never block further patch issues:
                #  dx=0: X=0 reads hit j = PBASE+64Y    -> zero [PBASE::64]
                #  dx=2: X=63 reads hit j = PBASE+63+64Y -> zero [PBASE+63::64]
                pT, pO, _ = pat_loc(p)
                nc.vector.memset(
                    bass.AP(
                        tensor=pT,
                        offset=pO + PBASE,
                        ap=[[PATW, 6], [64, 64]],
                    ).bitcast(F32),
                    0.0,
                )
                # partition base 12 is not DVE/gpsimd-writable (32-alignment
                # rule) -> zero-copy DMA for the dx=2 group.  MUST be HWDGE
                # (sync/scalar): SWDGE generates the 384 4B descriptors
                # serially on Q7 (~12us); HWDGE does it in RTL and spreads
                # them across all 16 SDMA slots (~1us).
                nc.sync.dma_start(
                    out=bass.AP(
                        tensor=pT,
                        offset=pO + 12 * PATW + PBASE + 63,
                        ap=[[PATW, 6], [64, 64]],
                    ),
                    in_=bass.AP(
                        tensor=zsrc[:].tensor,
                        offset=zsrc[:].offset,
                        ap=[[64, 6], [1, 64]],
                    ),
                )

            patch_pair(0)

            # stg2: conv2 padded staging rows (one partition per sample)
            stg2 = const.tile([8, 21, 19], F32R, tag="stg2")
            nc.vector.memset(stg2[:].bitcast(F32), 0.0)

            # ---------------- weight staging DMAs ---------------------------
            # s9x[c, 2q+s] (q = 3dx+dy) holds w_conv0[c%64, dy, dx] in the
            # (c<64) == (s==0) half, zero elsewhere; one PE transpose then
            # yields the conv0 lhsT block directly.  Staged via a t-order
            # load + 6 small strided gathers (all partition-step-1 APs).
            # s9t + gathers go on HWDGE (sync/scalar): as many-small-
            # descriptor DMAs they would serialize SWDGE's Q7 descriptor
            # generation for ~15us and gate ct2_c0 (and so conv0).
            s9t = const.tile([64, 9], F32, tag="s9t")
            nc.scalar.dma_start(out=s9t[:], in_=wc0[:].rearrange("a b c d -> a (b c d)"))
            s9x = const.tile([128, 18], F32, tag="s9x")
            nc.vector.memset(s9x[:], 0.0)
            for s in range(2):
                for dx in range(3):
                    nc.scalar.dma_start(
                        out=bass.AP(
                            tensor=s9x[:].tensor,
                            offset=s9x[:].offset + s * (64 * 18) + 3 * s + 6 * dx,
                            ap=[[18, 64], [1, 3]],
                        ),
                        in_=bass.AP(
                            tensor=s9t[:].tensor,
                            offset=s9t[:].offset + dx,
                            ap=[[9, 64], [3, 3]],
                        ),
                    )
            def load_wsrc(wdram):
                wsrc = dbl.tile([64, 576], F32, tag="wsrc")
                nc.gpsimd.dma_start(
                    out=wsrc[:], in_=wdram[:].rearrange("a b c d -> a (b c d)")
                )
                return wsrc

            # ---------------- conv biases (bt0/bt1 needed by the encoder
            # evictions; the rest load after the patch DMAs) ---------------
            def bias128(dram_b, tag, eng):
                bt = const.tile([128, 1], F32, tag=tag)
                eng.dma_start(out=bt[0:64, :], in_=dram_b[:].unsqueeze(1))
                eng.dma_start(out=bt[64:128, :], in_=dram_b[:].unsqueeze(1))
                return bt

            patch_pair(1)
            patch_pair(2)
            patch_pair(3)
            bt0 = bias128(bc0, "bt0", nc.gpsimd)
            bt1 = bias128(bc1, "bt1", nc.gpsimd)
            bt0s = const.tile([128, 1], F32, tag="bt0s")
            nc.vector.tensor_scalar_mul(bt0s[:], bt0[:], 0.25)
            bt1s = const.tile([128, 1], F32, tag="bt1s")
            nc.vector.tensor_scalar_mul(bt1s[:], bt1[:], 0.25)
            # pair-0's wrap zero issues right after the s9x gathers so
            # conv0 can start at ~16us; later pairs' zeros interleave with
            # the remaining weight loads.
            if wrap_zero:
                wrap_pair(0)
                wrap_pair(1)
            wsrc_c1 = load_wsrc(wc1)
            if wrap_zero:
                wrap_pair(2)
                wrap_pair(3)
            # ---------------- identity (for PE transposes) ------------------
            ident = const.tile([128, 128], F32, tag="ident")
            make_identity(nc, ident)

            # ---------------- PE warmup -------------------------------------
            # The HAM clock gate holds the PE at 1.2GHz until it has been
            # busy for a full 3.4us window.  While the patch DMAs stage,
            # keep the PE streaming dummy matmuls so the encoder starts at
            # 2.4GHz instead of half clock.
            wgarb = const.tile([128, 512], F32R, tag="wgarb")
            nc.vector.memset(wgarb[:].bitcast(F32), 0.0)

            def warm(n):
                # N=512 dummies: 213ns of guaranteed-dense PE busy each, so
                # a burst of ~16 always covers the 3.4us HAM re-warm window
                for _ in range(n):
                    pwu = psmm.tile([64, 512], F32, tag="mm")
                    nc.tensor.matmul(
                        pwu[:], wgarb[:, 0:64], wgarb[:], start=True, stop=True
                    )

            warm(62)

            # ---------------- 1ch conv weights ------------------------------
            # staged t-order [9, 64] via PE transpose, then scattered to the
            # block-diagonal replicated lhsT tiles with one remap DMA each.
            ct2_c0 = const.tile([128, 128], F32R, tag="ct2_c0")
            nc.vector.memset(ct2_c0[:].bitcast(F32), 0.0)
            ct2_c2 = const.tile([128, 128], F32R, tag="ct2_c2")
            nc.vector.memset(ct2_c2[:].bitcast(F32), 0.0)

            p9 = psmm.tile([18, 128], F32, tag="mm")
            nc.tensor.transpose(p9[:], s9x[:], ident[0:128, 0:128])
            nc.scalar.activation(
                ct2_c0[0:18, :], p9[:], AF.Copy, bias=0.0, scale=1.0
            )
            # replicas on SCALAR: they depend on the ct2_c0 ACT just above
            # (same queue, perfect ordering); on sync they sat at slots
            # 11-12 behind all patches+zeros and gated conv0(1/2) at ~34us
            for p in (1, 2):
                nc.scalar.dma_start(
                    out=ct2_c0[32 * p : 32 * p + 18, :],
                    in_=ct2_c0[0:18, :],
                )


            # ---------------- 64ch conv weights -> block-diag lhsT ----------
            wtap = {}

            def build_wtap(name, wsrc):
                wt = const.tile([128, 9, 128], B16, tag=f"wtap_{name}")
                nc.vector.memset(wt[:], 0.0)
                for t in range(9):
                    pw = psmm.tile([64, 64], F32, tag="mm")
                    nc.tensor.transpose(pw[:], wsrc[:, t::9], ident[0:64, 0:64])
                    nc.scalar.activation(
                        wt[0:64, t, 0:64], pw[:], AF.Copy, bias=0.0, scale=1.0
                    )
                nc.scalar.dma_start(out=wt[64:128, :, 64:128], in_=wt[0:64, :, 0:64])
                wtap[name] = wt

            build_wtap("c1", wsrc_c1)

            # enc conv weights (64ci -> 1co): one gather DMA, no scaling.
            encT = const.tile([128, 9, 2], F32R, tag="encT")
            nc.vector.memset(encT[:].bitcast(F32), 0.0)
            # out (c+64s)*18 + 2t + s  <-  wen flat c*9 + t  (one DMA per s)
            for s in range(2):
                nc.scalar.dma_start(
                    out=bass.AP(
                        tensor=encT[:].tensor,
                        offset=encT[:].offset + s * (64 * 18 + 1),
                        ap=[[18, 64], [2, 9]],
                    ),
                    in_=bass.AP(
                        tensor=wen[:].tensor,
                        offset=0,
                        ap=[[9, 64], [1, 9]],
                    ),
                )

            bte = const.tile([2, 1], F32, tag="bte")
            nc.sync.dma_start(
                out=bte[:],
                in_=bass.AP(tensor=ben[:].tensor, offset=0, ap=[[0, 2], [1, 1]]),
            )

            xstage = const.tile([8, 16, 16], F32, tag="xstage")

            # ================ encoder: interleaved over 4 sample pairs ======
            c1in_l = [None] * NPAIR
            ein_l = [None] * NPAIR

            def conv0_pair(p):
                c1in = tri3.tile([128, 34, 34], B16, tag="c1in")
                nc.gpsimd.memset(c1in[:, 0:1, :], 0.0)
                nc.gpsimd.memset(c1in[:, 33:34, :], 0.0)
                nc.gpsimd.memset(c1in[:, 1:33, 0:1], 0.0)
                nc.gpsimd.memset(c1in[:, 1:33, 33:34], 0.0)
                base = 32 * p if p < 3 else 64
                _, _, ptile = pat_loc(p)
                for n in range(8):
                    ps = psmm.tile([128, 4, 2, 32, 2], F32, tag="mm")
                    nc.tensor.matmul(
                        ps[:].rearrange("p a b c d -> p (a b c d)"),
                        ct2_c0[base : base + 18, :],
                        ptile[base : base + 18, PBASE + 512 * n : PBASE + 512 * (n + 1)],
                        start=True,
                        stop=True,
                    )
                    ct0 = trip.tile([128, 4, 2, 32, 2], F32, tag="ct0")
                    nc.scalar.activation(ct0[:], ps[:], AF.Relu, bias=bt0s, scale=0.25)
                    tcol = tri3.tile([128, 4, 2, 32], F32, tag="tcol")
                    nc.vector.tensor_add(
                        tcol[:], ct0[:, :, :, :, 0], ct0[:, :, :, :, 1]
                    )
                    nc.vector.tensor_add(
                        c1in[:, 1 + 4 * n : 5 + 4 * n, 1:33],
                        tcol[:, :, 0, :],
                        tcol[:, :, 1, :],
                    )
                c1in_l[p] = c1in

            def conv1_pair(p):
                c1in = c1in_l[p]
                e_in = quad.tile([128, 18, 18], F32R, tag="e_in")
                nc.gpsimd.memset(e_in[:, 0:1, :].bitcast(F32), 0.0)
                nc.gpsimd.memset(e_in[:, 17:18, :].bitcast(F32), 0.0)
                nc.gpsimd.memset(e_in[:, 1:17, 0:1].bitcast(F32), 0.0)
                nc.gpsimd.memset(e_in[:, 1:17, 17:18].bitcast(F32), 0.0)
                for n in range(2):
                    ps = psmm.tile([128, 8, 2, 16, 2], F32, tag="mm")
                    for t, (dy, dx) in enumerate(TAPS):
                        nc.tensor.matmul(
                            ps[:],
                            wtap["c1"][:, t, :],
                            c1in[:, n * 16 + dy : n * 16 + dy + 16, dx : dx + 32],
                            start=(t == 0),
                            stop=(t == 8),
                        )
                    ct1 = trip.tile([128, 8, 2, 16, 2], F32, tag="ct1")
                    nc.scalar.activation(ct1[:], ps[:], AF.Relu, bias=bt1s, scale=0.25)
                    tc1 = tri3.tile([128, 8, 2, 16], F32, tag="tc1")
                    nc.vector.tensor_add(
                        tc1[:], ct1[:, :, :, :, 0], ct1[:, :, :, :, 1]
                    )
                    nc.vector.tensor_add(
                        e_in[:, 1 + 8 * n : 9 + 8 * n, 1:17],
                        tc1[:, :, 0, :],
                        tc1[:, :, 1, :],
                    )
                ein_l[p] = e_in

            def enc_pair(p):
                e_in = ein_l[p]
                pe = psmm.tile([2, 16, 16], F32, tag="mm")
                for t, (dy, dx) in enumerate(TAPS):
                    nc.tensor.matmul(
                        pe[:],
                        encT[:, t, :],
                        e_in[:, dy : dy + 16, dx : dx + 16],
                        start=(t == 0),
                        stop=(t == 8),
                    )
                estage = dbl.tile([2, 16, 16], F32, tag="estage")
                nc.scalar.activation(estage[:], pe[:], AF.Relu, bias=bte)
                nc.scalar.dma_start(out=xstage[2 * p : 2 * p + 2, :, :], in_=estage[:])

            conv0_pair(0)
            if debug:
                nc.sync.dma_start(out=dbg["dbg_pat"][:], in_=pat[0:18, :])
                nc.sync.dma_start(out=dbg["dbg_ct2"][:], in_=ct2_c0[:])
                nc.sync.dma_start(out=dbg["dbg_c1in"][:], in_=c1in_l[0][:])
            conv0_pair(1)
            warm(10)
            conv1_pair(0)
            warm(1)
            conv0_pair(2)
            enc_pair(0)
            conv1_pair(1)
            warm(1)
            conv0_pair(3)
            enc_pair(1)
            conv1_pair(2)
            warm(1)
            enc_pair(2)
            conv1_pair(3)
            warm(1)
            enc_pair(3)

            # ---------------- deferred weight staging (decoder + NTM): -----
            # emitted after the encoder so these HBM loads queue BEHIND the
            # conv0 patch DMAs and don't stall the pipeline start; they
            # drain during the encoder compute.
            # s9y[c, 9s+t] = w_conv2[c%64, t] in the matching half (t-order).
            s9y = const.tile([128, 18], F32, tag="s9y")
            nc.vector.memset(s9y[:], 0.0)
            for s in range(2):
                nc.scalar.dma_start(
                    out=bass.AP(
                        tensor=s9y[:].tensor,
                        offset=s9y[:].offset + s * (64 * 18 + 9),
                        ap=[[18, 64], [1, 9]],
                    ),
                    in_=wc2[:].rearrange("a b c d -> a (b c d)"),
                )
            p9b = psmm.tile([18, 128], F32, tag="mm")
            nc.tensor.transpose(p9b[:], s9y[:], ident[0:128, 0:128])
            nc.scalar.activation(
                ct2_c2[0:18, :], p9b[:], AF.Copy, bias=0.0, scale=1.0
            )
            for p in (1, 2):
                nc.scalar.dma_start(
                    out=ct2_c2[32 * p : 32 * p + 18, :],
                    in_=ct2_c2[0:18, :],
                )
            bt2 = bias128(bc2, "bt2", nc.scalar)
            bt3 = bias128(bc3, "bt3", nc.sync)
            bt4 = bias128(bc4, "bt4", nc.scalar)

            # NTM weights: w_lstm_x rows 0:256, gate cols i/g/o
            wx = const.tile([128, 2, 768], F32R, tag="wx")
            for kt in range(2):
                nc.gpsimd.dma_start(
                    out=wx[:, kt, 0:256],
                    in_=wlx[kt * 128 : (kt + 1) * 128, 0:256],
                )
                nc.gpsimd.dma_start(
                    out=wx[:, kt, 256:768],
                    in_=wlx[kt * 128 : (kt + 1) * 128, 512:1024],
                )
            bigo = const.tile([128, 6], F32, tag="bigo")
            # cols (2j+h2): j in (i,g,o) -> b_lstm[0:256] and b_lstm[512:1024]
            nc.sync.dma_start(
                out=bass.AP(tensor=bigo[:].tensor, offset=bigo[:].offset,
                            ap=[[6, 128], [1, 2]]),
                in_=bass.AP(tensor=bls[:].tensor, offset=0,
                            ap=[[1, 128], [128, 2]]),
            )
            nc.sync.dma_start(
                out=bass.AP(tensor=bigo[:].tensor, offset=bigo[:].offset + 2,
                            ap=[[6, 128], [1, 4]]),
                in_=bass.AP(tensor=bls[:].tensor, offset=512,
                            ap=[[1, 128], [128, 4]]),
            )
            # w_out rows 0:256 (h part) + bias row
            wo = const.tile([128, 2, 256], F32R, tag="wo")
            nc.gpsimd.dma_start(out=wo[:, 0, :], in_=wou[0:128, :])
            nc.gpsimd.dma_start(out=wo[:, 1, :], in_=wou[128:256, :])
            rhs_b = const.tile([1, 256], F32R, tag="rhs_b")
            nc.scalar.dma_start(out=rhs_b[:], in_=bou[:].unsqueeze(0))
            ones1 = const.tile([1, 8], F32R, tag="ones1")
            nc.vector.memset(ones1[:].bitcast(F32), 1.0)

            # deferred weight prep: FOLDED decoder kernels (fills the PE
            # bubble while the NTM chain runs).  conv3/conv4 consume a 2x
            # nearest-neighbor-upsampled input, so conv(up2(X)) collapses
            # into 4 output-phase convolutions with 2x2 kernels over the
            # un-upsampled X: phase (a,b) kernel (ey,ex) = sum of W[dy,dx]
            # over dy in GRP[a][ey], dx in GRP[b][ex].  The tap sums are
            # accumulated directly in PSUM by the transposes.
            GRP = (((0,), (1, 2)), ((0, 1), (2,)))
            wfold = {}

            def build_wfold(name, wsrc):
                wf = const.tile([128, 16, 128], B16, tag=f"wfold_{name}")
                nc.vector.memset(wf[:], 0.0)
                for a in range(2):
                    for b in range(2):
                        for ey in range(2):
                            for ex in range(2):
                                k = 8 * a + 4 * b + 2 * ey + ex
                                taps = [
                                    3 * dy + dx
                                    for dy in GRP[a][ey]
                                    for dx in GRP[b][ex]
                                ]
                                pw = psmm.tile([64, 64], F32, tag="mm")
                                for i, t in enumerate(taps):
                                    nc.tensor.matmul(
                                        pw[:],
                                        wsrc[:, t::9],
                                        ident[0:64, 0:64],
                                        is_transpose=True,
                                        start=(i == 0),
                                        stop=(i == len(taps) - 1),
                                    )
                                nc.scalar.activation(
                                    wf[0:64, k, 0:64], pw[:], AF.Copy,
                                    bias=0.0, scale=1.0,
                                )
                nc.sync.dma_start(out=wf[64:128, :, 64:128], in_=wf[0:64, :, 0:64])
                wfold[name] = wf

            wsrc_c3 = load_wsrc(wc3)
            build_wfold("c3", wsrc_c3)
            wsrc_c4 = load_wsrc(wc4)
            build_wfold("c4", wsrc_c4)
            # bridge the gap while the enc evictions land in xstage
            warm(5)

            # ================ NTM step (all 8 samples at once) ==============
            if debug:
                nc.sync.dma_start(out=dbg["dbg_x"][:], in_=xstage[:])
            # x^T k-tiles via PE transpose
            xT = work.tile([128, 2, 8], F32R, tag="xT")
            for kt in range(2):
                pxt = psmm.tile([128, 8], F32, tag="mm")
                nc.tensor.transpose(
                    pxt[:],
                    xstage[:].rearrange("p a b -> p (a b)")[:, kt * 128 : kt * 128 + 128],
                    ident[0:8, 0:8],
                )
                nc.scalar.activation(xT[:, kt, :], pxt[:], AF.Copy, bias=0.0, scale=1.0)
            # z = x @ Wx + b for gates i, g, o; h = sig(o) * tanh(sig(i)*tanh(g))
            zps = psmm.tile([128, 6, 8], F32, tag="mm")
            for j in range(3):
                for h2 in range(2):
                    for kt in range(2):
                        nc.tensor.matmul(
                            zps[:, 2 * j + h2, :],
                            wx[:, kt, j * 256 + h2 * 128 : j * 256 + h2 * 128 + 128],
                            xT[:, kt, :],
                            start=(kt == 0),
                            stop=(kt == 1),
                        )
            zb = work.tile([128, 6, 8], F32, tag="zb")
            bigo_b = bass.AP(
                tensor=bigo[:].tensor, offset=bigo[:].offset,
                ap=[list(d) for d in bigo[:].ap] + [[0, 8]],
            )
            nc.vector.tensor_tensor(zb[:], zps[:], bigo_b, op=ALU.add)
            si = work.tile([128, 2, 8], F32, tag="gate0")
            nc.scalar.activation(si[:], zb[:, 0:2, :], AF.Sigmoid, bias=0.0)
            tg = work.tile([128, 2, 8], F32, tag="gate1")
            nc.scalar.activation(tg[:], zb[:, 2:4, :], AF.Tanh, bias=0.0)
            so = work.tile([128, 2, 8], F32, tag="gate2")
            nc.scalar.activation(so[:], zb[:, 4:6, :], AF.Sigmoid, bias=0.0)
            ctile = work.tile([128, 2, 8], F32, tag="ctile")
            nc.vector.tensor_mul(ctile[:], si[:], tg[:])
            tct = work.tile([128, 2, 8], F32, tag="tct")
            nc.scalar.activation(tct[:], ctile[:], AF.Tanh, bias=0.0)
            h = work.tile([128, 2, 8], F32R, tag="h")
            nc.vector.tensor_mul(h[:], so[:], tct[:])
            if debug:
                nc.sync.dma_start(out=dbg["dbg_h"][:], in_=h[:])
            # out = clip(h @ w_out[:256] + b_out)  (reads contribution dropped)
            pout = psmm.tile([8, 16, 16], F32, tag="mm")
            for kt in range(2):
                nc.tensor.matmul(
                    pout[:].rearrange("p a b -> p (a b)"),
                    h[:, kt, :],
                    wo[:, kt, :],
                    start=(kt == 0),
                    stop=False,
                )
            nc.tensor.matmul(
                pout[:].rearrange("p a b -> p (a b)"),
                ones1[:],
                rhs_b[:],
                start=False,
                stop=True,
            )
            nc.vector.tensor_scalar(
                stg2[:, 1:17, 1:17], pout[:], -CLIP, CLIP, ALU.max, ALU.min
            )
            # keep the PE clock warm while the NTM result fans out through
            # stg2 -> pc2 staging DMAs: the HAM gate needs a DENSE ~3.4us
            # busy window to hold/raise 2.4GHz, and a cold decoder start
            # costs ~15us.
            warm(18)
            if debug:
                nc.sync.dma_start(out=dbg["dbg_clip"][:], in_=stg2[:, 1:17, 1:17])

            # ================ decoder: stage-major over 4 pairs =============
            # conv2 patches for all pairs in one merged tile + one DMA.
            # partition r = 32p + 9s + 3dy + dx via the overlapping stride-1
            # dx trick (reads stg2 shifted by 0/1/2 columns).
            pc2 = const.tile([128, 684], F32R, tag="pc2")
            for p in range(NPAIR):
                base = 32 * p if p < 3 else 0
                c0 = 0 if p < 3 else 342
                for s in range(2):
                    for dy in range(3):
                        eng = dmaeng[(2 * p + s + dy) % 3]
                        eng.dma_start(
                            out=bass.AP(
                                tensor=pc2[:].tensor,
                                offset=pc2[:].offset
                                + (base + 9 * s + 3 * dy) * 684 + c0,
                                ap=[[684, 3], [1, 341]],
                            ),
                            in_=bass.AP(
                                tensor=stg2[:].tensor,
                                offset=stg2[:].offset + (2 * p + s) * 399 + dy * 19,
                                ap=[[399, 1], [1, 3], [1, 341]],
                            ),
                        )

            # --- conv2 all pairs -> padded stage S2 (reuses the quad pool
            # slots freed by the enc pairs; 1 eviction per pair instead of
            # the old 4 upsample writes)
            c2s_l = []
            for p in range(NPAIR):
                base = 32 * p if p < 3 else 0
                c0 = 0 if p < 3 else 342
                ps2 = psmm.tile([128, 16, 16], F32, tag="mm")
                nc.tensor.matmul(
                    ps2[:],
                    ct2_c2[base : base + 18, :],
                    pc2[base : base + 18, c0 : c0 + 342]
                    .rearrange("p (a b) -> p a b", a=18)[:, 0:16, 0:16],
                    start=True,
                    stop=True,
                )
                S2 = quad.tile([128, 18, 18], B16, tag="c2s")
                nc.gpsimd.memset(S2[:, 0:1, :], 0.0)
                nc.gpsimd.memset(S2[:, 17:18, :], 0.0)
                nc.gpsimd.memset(S2[:, 1:17, 0:1], 0.0)
                nc.gpsimd.memset(S2[:, 1:17, 17:18], 0.0)
                nc.scalar.activation(S2[:, 1:17, 1:17], ps2[:], AF.Relu, bias=bt2)
                c2s_l.append(S2)
            if debug:
                nc.sync.dma_start(out=dbg["dbg_pc2"][:], in_=pc2[:])
                nc.sync.dma_start(out=dbg["dbg_ctc2"][:], in_=ct2_c2[:])

            # --- conv3 folded: 4 output-phase 2x2 convs over S2 -> S3
            # (34x34 padded, UN-upsampled conv3 output)
            c3s_l = [None] * NPAIR

            def conv3_pair(p):
                S2 = c2s_l[p]
                S3 = c3p.tile([128, 34, 34], B16, tag="c3s")
                nc.gpsimd.memset(S3[:, 0:1, :], 0.0)
                nc.gpsimd.memset(S3[:, 33:34, :], 0.0)
                nc.gpsimd.memset(S3[:, 1:33, 0:1], 0.0)
                nc.gpsimd.memset(S3[:, 1:33, 33:34], 0.0)
                S3v = S3[:].rearrange(
                    "p (ri ra) (ci cb) -> p ri ra ci cb", ra=2, cb=2
                )
                for a in range(2):
                    for b in range(2):
                        ps = psc3.tile([128, 16, 16], F32, tag="mm3")
                        i = 0
                        for ey in range(2):
                            for ex in range(2):
                                k = 8 * a + 4 * b + 2 * ey + ex
                                nc.tensor.matmul(
                                    ps[:],
                                    wfold["c3"][:, k, :],
                                    S2[:, a + ey : a + ey + 16,
                                       b + ex : b + ex + 16],
                                    start=(i == 0),
                                    stop=(i == 3),
                                )
                                i += 1
                        # out row 1+2r+a, col 1+2c+b in S3
                        rs = slice(0, 16) if a == 0 else slice(1, 17)
                        cs = slice(0, 16) if b == 0 else slice(1, 17)
                        dst = S3v[:, rs, 1 - a, cs, 1 - b]
                        if (a + b) % 2 == 0:
                            nc.scalar.activation(dst, ps[:], AF.Relu, bias=bt3)
                        else:
                            nc.vector.tensor_scalar(
                                dst, ps[:], bt3[:], 0.0, ALU.add, ALU.max
                            )
                c3s_l[p] = S3

            # --- conv4 folded: phases over S3; each 32-row block stores as
            # one 1MB DMA as soon as its 4 phases are evicted.
            store_rot = [nc.gpsimd, nc.sync]

            def conv4_pair(p):
                S3 = c3s_l[p]
                c4out = out2.tile([128, 64, 64], F32, tag="c4out")
                c4v = c4out[:].rearrange(
                    "p (ri ra) (ci cb) -> p ri ra ci cb", ra=2, cb=2
                )
                for h in range(2):
                    for a in range(2):
                        for b in range(2):
                            ps = psmm.tile([128, 16, 32], F32, tag="mm")
                            i = 0
                            for ey in range(2):
                                for ex in range(2):
                                    k = 8 * a + 4 * b + 2 * ey + ex
                                    nc.tensor.matmul(
                                        ps[:],
                                        wfold["c4"][:, k, :],
                                        S3[:, 16 * h + a + ey : 16 * h + a + ey + 16,
                                           b + ex : b + ex + 32],
                                        start=(i == 0),
                                        stop=(i == 3),
                                    )
                                    i += 1
                            dst = c4v[:, 16 * h : 16 * h + 16, a, 0:32, b]
                            if (a + b) % 2 == 0:
                                nc.scalar.activation(dst, ps[:], AF.Relu, bias=bt4)
                            else:
                                nc.vector.tensor_scalar(
                                    dst, ps[:], bt4[:], 0.0, ALU.add, ALU.max
                                )
                    store_rot[(2 * p + h) % 2].dma_start(
                        out=bass.AP(
                            tensor=out[:].tensor,
                            offset=2 * p * 262144 + 2048 * h,
                            ap=[[4096, 128], [1, 2048]],
                        ),
                        in_=bass.AP(
                            tensor=c4out[:].tensor,
                            offset=c4out[:].offset + 2048 * h,
                            ap=[[4096, 128], [1, 2048]],
                        ),
                    )

            conv3_pair(0)
            conv3_pair(1)
            conv4_pair(0)
            conv3_pair(2)
            conv4_pair(1)
            conv3_pair(3)
            conv4_pair(2)
            conv4_pair(3)

    nc.compile()
    return nc


_NC_CACHE = {}
LAST_RESULT = None

WEIGHT_NAMES = [
    "w_conv0", "b_conv0", "w_conv1", "b_conv1", "w_enc", "b_enc",
    "w_conv2", "b_conv2", "w_conv3", "b_conv3", "w_conv4", "b_conv4",
    "w_lstm_x", "b_lstm", "w_out", "b_out",
]


def kernel(**inputs):
    global LAST_RESULT
    from concourse.bass_utils import run_bass_kernel_spmd

    debug = bool(int(os.environ.get("KDEBUG", "0")))
    key = ("nc", debug)
    if key not in _NC_CACHE:
        _NC_CACHE[key] = build_nc(debug=debug)
    nc = _NC_CACHE[key]

    xs = np.ascontiguousarray(np.asarray(inputs["inputs"], dtype=np.float32))
    weights = {
        k: np.ascontiguousarray(np.asarray(inputs[k], dtype=np.float32))
        for k in WEIGHT_NAMES
    }
    in_maps = []
    for c in range(N_CORES):
        m = dict(weights)
        m["inputs"] = xs[c * B_CORE : (c + 1) * B_CORE]
        in_maps.append(m)

    res = run_bass_kernel_spmd(nc, in_maps, core_ids=list(range(N_CORES)))
    LAST_RESULT = res
    return np.concatenate([r["out"] for r in res.results], axis=0)


if __name__ == "__main__":
    nc = build_nc()
    print("built ok")



# revision 84
# speedup vs baseline: 1.0307x; 1.0014x over previous
"""Trainium2 Bass kernel for nn_Encoder_Decoder_Wrapper (conv encoder -> NTM step -> conv decoder).

Sharding: pure data parallel, batch 64 -> 8 cores x 8 samples; weights
replicated.  Per core, samples run in 4 pairs of 2 so every 64-channel conv
is a K=128/M=128 block-diagonal matmul (2 samples packed in contraction and
output partitions).

conv0 patch staging: 9 tap-shifted image copies per pair, partition order
r = dy + 3s + 6dx, so one 96KB HBM DMA per (pair, dy) fills a stride-3
partition set with the (s, dx) tap shifts expressed as linear source axes
(12 DMAs total).  dx=0 row-wrap elements are killed by an aligned DVE
strided memset, dx=2 by an HWDGE zero-copy DMA (SWDGE would generate its
384 4-byte descriptors serially on Q7, ~12us).  Pair 3 sits at partitions
64-81 of a second tile to use the odd SDMA-engine set.  conv0 stays fp32r
(its rhs streams straight from the f32 input).

NTM step reduced via its constant initial state: reads0 = h0 = c0 = 0, so
only out = clip(h @ w_out[:256] + b_out) with h = sig(o)*tanh(sig(i)*tanh(g));
the dropped ~1e-6 read vectors change the output by ~2e-4 relative.

Decoder: conv3/conv4 consume 2x nearest-neighbor-upsampled inputs, so
conv(up2(X)) is FOLDED into 4 output-phase convolutions with 2x2 kernels
over the un-upsampled X (phase (a,b) kernel (ey,ex) = sum of W[dy,dx] over
dy in GRP[a][ey], dx in GRP[b][ex], summed in PSUM by the weight-prep
transposes): 2.25x fewer matmul columns and no upsample staging.  Each
32-row conv4 block stores as one 1MB single-partition-axis DMA as soon as
its 4 phases evict, fully overlapping the 8MB writeback.

conv1 and the decoder run in bfloat16 (weights + staged activations, fp32
PSUM accumulation): fp32r matmuls execute two-pass (fp32_mode=LOW_HIGH)
with a ~190ns fused 4-byte weight load that dominates short tiles; bf16
streams single-pass with half-size weight loads and doubles DVE eviction
throughput.  Total rel err ~6e-3 vs the 2e-2 budget.

Dummy N=512 matmul bursts ("warm") keep the PE HAM clock gate at 2.4GHz
through the patch-staging window and the NTM serial section: the gate
falls to 1.2GHz after ~3.4us idle and needs ~3.4us of dense busy to
recover, so each cold dip costs ~10us.  Bursts must be emitted BEFORE any
instruction that blocks on a slow load (engine queues are FIFO).
"""

import os
import sys

sys.path.insert(0, "/opt/trn_rl_repo")
os.environ.setdefault("MYCRO_LOCAL_CACHE", "1")

import numpy as np

import concourse.bass as bass
import concourse.bacc as bacc
import concourse.mybir as mybir
import concourse.tile as tile
from concourse.masks import make_identity

F32 = mybir.dt.float32
F32R = mybir.dt.float32r
B16 = mybir.dt.bfloat16
AF = mybir.ActivationFunctionType
ALU = mybir.AluOpType

TAPS = [(dy, dx) for dy in range(3) for dx in range(3)]
CLIP = 20.0

N_CORES = 8
B_CORE = 8          # samples per core
NPAIR = B_CORE // 2

PATW = 4240         # per-partition conv0 patch buffer (elements)
PBASE = 65          # read-window base offset: window j in [PBASE, PBASE+4096)


def build_nc(debug=False):
    nc = bacc.Bacc(None, target_bir_lowering=False)

    inp = nc.dram_tensor("inputs", [B_CORE, 1, 64, 64], F32R, kind="ExternalInput")
    wc0 = nc.dram_tensor("w_conv0", [64, 1, 3, 3], F32, kind="ExternalInput")
    bc0 = nc.dram_tensor("b_conv0", [64], F32, kind="ExternalInput")
    wc1 = nc.dram_tensor("w_conv1", [64, 64, 3, 3], F32, kind="ExternalInput")
    bc1 = nc.dram_tensor("b_conv1", [64], F32, kind="ExternalInput")
    wen = nc.dram_tensor("w_enc", [1, 64, 3, 3], F32R, kind="ExternalInput")
    ben = nc.dram_tensor("b_enc", [1], F32, kind="ExternalInput")
    wc2 = nc.dram_tensor("w_conv2", [64, 1, 3, 3], F32, kind="ExternalInput")
    bc2 = nc.dram_tensor("b_conv2", [64], F32, kind="ExternalInput")
    wc3 = nc.dram_tensor("w_conv3", [64, 64, 3, 3], F32, kind="ExternalInput")
    bc3 = nc.dram_tensor("b_conv3", [64], F32, kind="ExternalInput")
    wc4 = nc.dram_tensor("w_conv4", [64, 64, 3, 3], F32, kind="ExternalInput")
    bc4 = nc.dram_tensor("b_conv4", [64], F32, kind="ExternalInput")
    wlx = nc.dram_tensor("w_lstm_x", [1024, 1024], F32R, kind="ExternalInput")
    bls = nc.dram_tensor("b_lstm", [1024], F32, kind="ExternalInput")
    wou = nc.dram_tensor("w_out", [1024, 256], F32R, kind="ExternalInput")
    bou = nc.dram_tensor("b_out", [256], F32R, kind="ExternalInput")
    out = nc.dram_tensor("out", [B_CORE, 64, 64, 64], F32, kind="ExternalOutput")

    dbg = {}
    if debug:
        for name, shape, dt in [
            ("dbg_h", [128, 2, 8], F32R),
            ("dbg_clip", [B_CORE, 16, 16], F32R),
            ("dbg_x", [B_CORE, 16, 16], F32),
            ("dbg_pat", [18, PATW], F32R),
            ("dbg_ct2", [128, 128], F32R),
            ("dbg_c1in", [128, 34, 34], F32R),
            ("dbg_pc2", [128, 684], F32R),
            ("dbg_ctc2", [128, 128], F32R),
        ]:
            dbg[name] = nc.dram_tensor(name, shape, dt, kind="ExternalOutput")

    with tile.TileContext(nc) as tc:
        with (
            tc.tile_pool(name="const", bufs=1) as const,
            tc.tile_pool(name="work", bufs=1) as work,
            tc.tile_pool(name="dbl", bufs=2) as dbl,
            tc.tile_pool(name="trip", bufs=3) as trip,
            tc.tile_pool(name="tri3", bufs=3) as tri3,
            tc.tile_pool(name="quad", bufs=4) as quad,
            tc.tile_pool(name="c3p", bufs=3) as c3p,
            tc.tile_pool(name="out2", bufs=2) as out2,
            tc.tile_pool(name="psmm", bufs=6, space="PSUM") as psmm,
            tc.tile_pool(name="psc3", bufs=2, space="PSUM") as psc3,
        ):
            dmaeng = [nc.sync, nc.gpsimd, nc.scalar]
            wrap_zero = bool(int(os.environ.get("KWRAP", "1")))

            # ---------------- conv0 patch buffer + pad memsets --------------
            # pairs 0-2 at base partitions 0/32/64; pair 3 (base partition
            # 96 is not a legal matmul operand base) lives at base 0 of a
            # second column range.
            # pair 3 lives at partitions 64-81 of a second tile so its patch
            # DMAs land on the ODD SDMA-engine set (partitions 64+) and
            # balance against pairs 0/1 on the even set.
            pat = const.tile([128, PATW], F32R, tag="pat")
            pat3 = const.tile([128, PATW], F32R, tag="pat3")
            patT = pat[:].tensor
            patO = pat[:].offset
            pat3T = pat3[:].tensor
            pat3O = pat3[:].offset + 64 * PATW
            # top/bottom row pads of the read window (shared by all taps);
            # emitted BEFORE the patch DMAs so interiors get overwritten.
            nc.vector.memset(pat[:, PBASE : PBASE + 65].bitcast(F32), 0.0)
            nc.vector.memset(pat[:, PBASE + 4031 : PBASE + 4096].bitcast(F32), 0.0)
            nc.vector.memset(pat3[:, PBASE : PBASE + 65].bitcast(F32), 0.0)
            nc.vector.memset(pat3[:, PBASE + 4031 : PBASE + 4096].bitcast(F32), 0.0)

            def pat_loc(p):
                if p < 3:
                    return patT, patO + 32 * p * PATW, pat
                return pat3T, pat3O, pat3

            # ---------------- conv0 patch DMAs: issued FIRST ----------------
            # Patch partition order r = 6dy + 3s + dx.  For a fixed dy all 6
            # partitions share the same trimmed dst window [d0, d0+4094)
            # (d0 = 66 - 64(dy-1)), and the (s, dx) tap shifts become LINEAR
            # source axes [[4096, 2], [1, 3]], so ONE 96KB HBM DMA fills the
            # whole dy-group: 3 DMAs per pair instead of 9.  (DMA AP shapes
            # need not match across sides; only the final contiguous dim
            # must.)  The few corner elements the common trim loses read as
            # the zero pad: ~4 real pixels per sample, negligible.
            pengs = [nc.sync, nc.scalar]
            if wrap_zero:
                zsrc = const.tile([128, 64], F32R, tag="zsrc")
                nc.vector.memset(zsrc[:].bitcast(F32), 0.0)

            # Partition order r = dy + 3s + 6dx: each dy-group is the
            # stride-3 set {dy, dy+3, .., dy+15} (spans ~4 SDMA engines
            # instead of 2), dx=0 is partitions 0-5 (aligned -> DVE memset
            # wrap-zero) and dx=2 is partitions 12-17 (one gpsimd zero-DMA).
            def patch_pair(p):
                pT, pO, _ = pat_loc(p)
                for dy in range(3):
                    d0 = 66 - 64 * (dy - 1)
                    (nc.sync if p < 2 else nc.gpsimd).dma_start(
                        out=bass.AP(
                            tensor=pT,
                            offset=pO + dy * PATW + d0,
                            ap=[[3 * PATW, 6], [1, 4094]],
                        ),
                        in_=bass.AP(
                            tensor=inp[:].tensor,
                            offset=2 * p * 4096,
                            ap=[[1, 3], [4096, 2], [1, 4094]],
                        ),
                    )

            def wrap_pair(p):
                # wrap-element zeroing, emitted AFTER all patch DMAs so the
                # completion waits never block further patch issues:
                #  dx=0: X=0 reads hit j = PBASE+64Y    -> zero [PBASE::64]
                #  dx=2: X=63 reads hit j = PBASE+63+64Y -> zero [PBASE+63::64]
                pT, pO, _ = pat_loc(p)
                nc.vector.memset(
                    bass.AP(
                        tensor=pT,
                        offset=pO + PBASE,
                        ap=[[PATW, 6], [64, 64]],
                    ).bitcast(F32),
                    0.0,
                )
                # partition base 12 is not DVE/gpsimd-writable (32-alignment
                # rule) -> zero-copy DMA for the dx=2 group.  MUST be HWDGE
                # (sync/scalar): SWDGE generates the 384 4B descriptors
                # serially on Q7 (~12us); HWDGE does it in RTL and spreads
                # them across all 16 SDMA slots (~1us).
                nc.sync.dma_start(
                    out=bass.AP(
                        tensor=pT,
                        offset=pO + 12 * PATW + PBASE + 63,
                        ap=[[PATW, 6], [64, 64]],
                    ),
                    in_=bass.AP(
                        tensor=zsrc[:].tensor,
                        offset=zsrc[:].offset,
                        ap=[[64, 6], [1, 64]],
                    ),
                )

            patch_pair(0)

            # stg2: conv2 padded staging rows (one partition per sample)
            stg2 = const.tile([8, 21, 19], F32R, tag="stg2")
            nc.vector.memset(stg2[:].bitcast(F32), 0.0)

            # ---------------- weight staging DMAs ---------------------------
            # s9x[c, 2q+s] (q = 3dx+dy) holds w_conv0[c%64, dy, dx] in the
            # (c<64) == (s==0) half, zero elsewhere; one PE transpose then
            # yields the conv0 lhsT block directly.  Staged via a t-order
            # load + 6 small strided gathers (all partition-step-1 APs).
            # s9t + gathers go on HWDGE (sync/scalar): as many-small-
            # descriptor DMAs they would serialize SWDGE's Q7 descriptor
            # generation for ~15us and gate ct2_c0 (and so conv0).
            s9t = const.tile([64, 9], F32, tag="s9t")
            nc.scalar.dma_start(out=s9t[:], in_=wc0[:].rearrange("a b c d -> a (b c d)"))
            s9x = const.tile([128, 18], F32, tag="s9x")
            nc.vector.memset(s9x[:], 0.0)
            for s in range(2):
                for dx in range(3):
                    nc.scalar.dma_start(
                        out=bass.AP(
                            tensor=s9x[:].tensor,
                            offset=s9x[:].offset + s * (64 * 18) + 3 * s + 6 * dx,
                            ap=[[18, 64], [1, 3]],
                        ),
                        in_=bass.AP(
                            tensor=s9t[:].tensor,
                            offset=s9t[:].offset + dx,
                            ap=[[9, 64], [3, 3]],
                        ),
                    )
            def load_wsrc(wdram):
                wsrc = dbl.tile([64, 576], F32, tag="wsrc")
                nc.gpsimd.dma_start(
                    out=wsrc[:], in_=wdram[:].rearrange("a b c d -> a (b c d)")
                )
                return wsrc

            # ---------------- conv biases (bt0/bt1 needed by the encoder
            # evictions; the rest load after the patch DMAs) ---------------
            def bias128(dram_b, tag, eng):
                bt = const.tile([128, 1], F32, tag=tag)
                eng.dma_start(out=bt[0:64, :], in_=dram_b[:].unsqueeze(1))
                eng.dma_start(out=bt[64:128, :], in_=dram_b[:].unsqueeze(1))
                return bt

            patch_pair(1)
            patch_pair(2)
            patch_pair(3)
            bt0 = bias128(bc0, "bt0", nc.gpsimd)
            bt1 = bias128(bc1, "bt1", nc.gpsimd)
            bt0s = const.tile([128, 1], F32, tag="bt0s")
            nc.vector.tensor_scalar_mul(bt0s[:], bt0[:], 0.25)
            bt1s = const.tile([128, 1], F32, tag="bt1s")
            nc.vector.tensor_scalar_mul(bt1s[:], bt1[:], 0.25)
            # pair-0's wrap zero issues right after the s9x gathers so
            # conv0 can start at ~16us; later pairs' zeros interleave with
            # the remaining weight loads.
            # zero order 0,2,1,3: pairs 2/3 stage on gpsimd and finish
            # early, so their zeros shouldn't queue behind pair 1's wait
            if wrap_zero:
                wrap_pair(0)
                wrap_pair(2)
            wsrc_c1 = load_wsrc(wc1)
            if wrap_zero:
                wrap_pair(1)
                wrap_pair(3)
            # ---------------- identity (for PE transposes) ------------------
            ident = const.tile([128, 128], F32, tag="ident")
            make_identity(nc, ident)

            # ---------------- PE warmup -------------------------------------
            # The HAM clock gate holds the PE at 1.2GHz until it has been
            # busy for a full 3.4us window.  While the patch DMAs stage,
            # keep the PE streaming dummy matmuls so the encoder starts at
            # 2.4GHz instead of half clock.
            wgarb = const.tile([128, 512], F32R, tag="wgarb")
            nc.vector.memset(wgarb[:].bitcast(F32), 0.0)

            def warm(n):
                # N=512 dummies: 213ns of guaranteed-dense PE busy each, so
                # a burst of ~16 always covers the 3.4us HAM re-warm window
                for _ in range(n):
                    pwu = psmm.tile([64, 512], F32, tag="mm")
                    nc.tensor.matmul(
                        pwu[:], wgarb[:, 0:64], wgarb[:], start=True, stop=True
                    )

            warm(62)

            # ---------------- 1ch conv weights ------------------------------
            # staged t-order [9, 64] via PE transpose, then scattered to the
            # block-diagonal replicated lhsT tiles with one remap DMA each.
            ct2_c0 = const.tile([128, 128], F32R, tag="ct2_c0")
            nc.vector.memset(ct2_c0[:].bitcast(F32), 0.0)
            ct2_c2 = const.tile([128, 128], F32R, tag="ct2_c2")
            nc.vector.memset(ct2_c2[:].bitcast(F32), 0.0)

            p9 = psmm.tile([18, 128], F32, tag="mm")
            nc.tensor.transpose(p9[:], s9x[:], ident[0:128, 0:128])
            nc.scalar.activation(
                ct2_c0[0:18, :], p9[:], AF.Copy, bias=0.0, scale=1.0
            )
            # replicas on SCALAR: they depend on the ct2_c0 ACT just above
            # (same queue, perfect ordering); on sync they sat at slots
            # 11-12 behind all patches+zeros and gated conv0(1/2) at ~34us
            for p in (1, 2):
                nc.scalar.dma_start(
                    out=ct2_c0[32 * p : 32 * p + 18, :],
                    in_=ct2_c0[0:18, :],
                )


            # ---------------- 64ch conv weights -> block-diag lhsT ----------
            wtap = {}

            def build_wtap(name, wsrc):
                wt = const.tile([128, 9, 128], B16, tag=f"wtap_{name}")
                nc.vector.memset(wt[:], 0.0)
                for t in range(9):
                    pw = psmm.tile([64, 64], F32, tag="mm")
                    nc.tensor.transpose(pw[:], wsrc[:, t::9], ident[0:64, 0:64])
                    nc.scalar.activation(
                        wt[0:64, t, 0:64], pw[:], AF.Copy, bias=0.0, scale=1.0
                    )
                nc.scalar.dma_start(out=wt[64:128, :, 64:128], in_=wt[0:64, :, 0:64])
                wtap[name] = wt

            build_wtap("c1", wsrc_c1)

            # enc conv weights (64ci -> 1co): one gather DMA, no scaling.
            encT = const.tile([128, 9, 2], F32R, tag="encT")
            nc.vector.memset(encT[:].bitcast(F32), 0.0)
            # out (c+64s)*18 + 2t + s  <-  wen flat c*9 + t  (one DMA per s)
            for s in range(2):
                nc.scalar.dma_start(
                    out=bass.AP(
                        tensor=encT[:].tensor,
                        offset=encT[:].offset + s * (64 * 18 + 1),
                        ap=[[18, 64], [2, 9]],
                    ),
                    in_=bass.AP(
                        tensor=wen[:].tensor,
                        offset=0,
                        ap=[[9, 64], [1, 9]],
                    ),
                )

            bte = const.tile([2, 1], F32, tag="bte")
            nc.sync.dma_start(
                out=bte[:],
                in_=bass.AP(tensor=ben[:].tensor, offset=0, ap=[[0, 2], [1, 1]]),
            )

            xstage = const.tile([8, 16, 16], F32, tag="xstage")

            # ================ encoder: interleaved over 4 sample pairs ======
            c1in_l = [None] * NPAIR
            ein_l = [None] * NPAIR

            def conv0_pair(p):
                c1in = tri3.tile([128, 34, 34], B16, tag="c1in")
                nc.gpsimd.memset(c1in[:, 0:1, :], 0.0)
                nc.gpsimd.memset(c1in[:, 33:34, :], 0.0)
                nc.gpsimd.memset(c1in[:, 1:33, 0:1], 0.0)
                nc.gpsimd.memset(c1in[:, 1:33, 33:34], 0.0)
                base = 32 * p if p < 3 else 64
                _, _, ptile = pat_loc(p)
                for n in range(8):
                    ps = psmm.tile([128, 4, 2, 32, 2], F32, tag="mm")
                    nc.tensor.matmul(
                        ps[:].rearrange("p a b c d -> p (a b c d)"),
                        ct2_c0[base : base + 18, :],
                        ptile[base : base + 18, PBASE + 512 * n : PBASE + 512 * (n + 1)],
                        start=True,
                        stop=True,
                    )
                    ct0 = trip.tile([128, 4, 2, 32, 2], F32, tag="ct0")
                    nc.scalar.activation(ct0[:], ps[:], AF.Relu, bias=bt0s, scale=0.25)
                    tcol = tri3.tile([128, 4, 2, 32], F32, tag="tcol")
                    nc.vector.tensor_add(
                        tcol[:], ct0[:, :, :, :, 0], ct0[:, :, :, :, 1]
                    )
                    nc.vector.tensor_add(
                        c1in[:, 1 + 4 * n : 5 + 4 * n, 1:33],
                        tcol[:, :, 0, :],
                        tcol[:, :, 1, :],
                    )
                c1in_l[p] = c1in

            def conv1_pair(p):
                c1in = c1in_l[p]
                e_in = quad.tile([128, 18, 18], F32R, tag="e_in")
                nc.gpsimd.memset(e_in[:, 0:1, :].bitcast(F32), 0.0)
                nc.gpsimd.memset(e_in[:, 17:18, :].bitcast(F32), 0.0)
                nc.gpsimd.memset(e_in[:, 1:17, 0:1].bitcast(F32), 0.0)
                nc.gpsimd.memset(e_in[:, 1:17, 17:18].bitcast(F32), 0.0)
                for n in range(2):
                    ps = psmm.tile([128, 8, 2, 16, 2], F32, tag="mm")
                    for t, (dy, dx) in enumerate(TAPS):
                        nc.tensor.matmul(
                            ps[:],
                            wtap["c1"][:, t, :],
                            c1in[:, n * 16 + dy : n * 16 + dy + 16, dx : dx + 32],
                            start=(t == 0),
                            stop=(t == 8),
                        )
                    ct1 = trip.tile([128, 8, 2, 16, 2], F32, tag="ct1")
                    nc.scalar.activation(ct1[:], ps[:], AF.Relu, bias=bt1s, scale=0.25)
                    tc1 = tri3.tile([128, 8, 2, 16], F32, tag="tc1")
                    nc.vector.tensor_add(
                        tc1[:], ct1[:, :, :, :, 0], ct1[:, :, :, :, 1]
                    )
                    nc.vector.tensor_add(
                        e_in[:, 1 + 8 * n : 9 + 8 * n, 1:17],
                        tc1[:, :, 0, :],
                        tc1[:, :, 1, :],
                    )
                ein_l[p] = e_in

            def enc_pair(p):
                e_in = ein_l[p]
                pe = psmm.tile([2, 16, 16], F32, tag="mm")
                for t, (dy, dx) in enumerate(TAPS):
                    nc.tensor.matmul(
                        pe[:],
                        encT[:, t, :],
                        e_in[:, dy : dy + 16, dx : dx + 16],
                        start=(t == 0),
                        stop=(t == 8),
                    )
                estage = dbl.tile([2, 16, 16], F32, tag="estage")
                nc.scalar.activation(estage[:], pe[:], AF.Relu, bias=bte)
                nc.scalar.dma_start(out=xstage[2 * p : 2 * p + 2, :, :], in_=estage[:])

            conv0_pair(0)
            if debug:
                nc.sync.dma_start(out=dbg["dbg_pat"][:], in_=pat[0:18, :])
                nc.sync.dma_start(out=dbg["dbg_ct2"][:], in_=ct2_c0[:])
                nc.sync.dma_start(out=dbg["dbg_c1in"][:], in_=c1in_l[0][:])
            conv0_pair(1)
            warm(10)
            conv1_pair(0)
            warm(1)
            conv0_pair(2)
            enc_pair(0)
            conv1_pair(1)
            warm(1)
            conv0_pair(3)
            enc_pair(1)
            conv1_pair(2)
            warm(1)
            enc_pair(2)
            conv1_pair(3)
            warm(1)
            enc_pair(3)

            # ---------------- deferred weight staging (decoder + NTM): -----
            # emitted after the encoder so these HBM loads queue BEHIND the
            # conv0 patch DMAs and don't stall the pipeline start; they
            # drain during the encoder compute.
            # s9y[c, 9s+t] = w_conv2[c%64, t] in the matching half (t-order).
            s9y = const.tile([128, 18], F32, tag="s9y")
            nc.vector.memset(s9y[:], 0.0)
            for s in range(2):
                nc.scalar.dma_start(
                    out=bass.AP(
                        tensor=s9y[:].tensor,
                        offset=s9y[:].offset + s * (64 * 18 + 9),
                        ap=[[18, 64], [1, 9]],
                    ),
                    in_=wc2[:].rearrange("a b c d -> a (b c d)"),
                )
            p9b = psmm.tile([18, 128], F32, tag="mm")
            nc.tensor.transpose(p9b[:], s9y[:], ident[0:128, 0:128])
            nc.scalar.activation(
                ct2_c2[0:18, :], p9b[:], AF.Copy, bias=0.0, scale=1.0
            )
            for p in (1, 2):
                nc.scalar.dma_start(
                    out=ct2_c2[32 * p : 32 * p + 18, :],
                    in_=ct2_c2[0:18, :],
                )
            bt2 = bias128(bc2, "bt2", nc.scalar)
            bt3 = bias128(bc3, "bt3", nc.sync)
            bt4 = bias128(bc4, "bt4", nc.scalar)

            # NTM weights: w_lstm_x rows 0:256, gate cols i/g/o
            wx = const.tile([128, 2, 768], F32R, tag="wx")
            for kt in range(2):
                nc.gpsimd.dma_start(
                    out=wx[:, kt, 0:256],
                    in_=wlx[kt * 128 : (kt + 1) * 128, 0:256],
                )
                nc.gpsimd.dma_start(
                    out=wx[:, kt, 256:768],
                    in_=wlx[kt * 128 : (kt + 1) * 128, 512:1024],
                )
            bigo = const.tile([128, 6], F32, tag="bigo")
            # cols (2j+h2): j in (i,g,o) -> b_lstm[0:256] and b_lstm[512:1024]
            nc.sync.dma_start(
                out=bass.AP(tensor=bigo[:].tensor, offset=bigo[:].offset,
                            ap=[[6, 128], [1, 2]]),
                in_=bass.AP(tensor=bls[:].tensor, offset=0,
                            ap=[[1, 128], [128, 2]]),
            )
            nc.sync.dma_start(
                out=bass.AP(tensor=bigo[:].tensor, offset=bigo[:].offset + 2,
                            ap=[[6, 128], [1, 4]]),
                in_=bass.AP(tensor=bls[:].tensor, offset=512,
                            ap=[[1, 128], [128, 4]]),
            )
            # w_out rows 0:256 (h part) + bias row
            wo = const.tile([128, 2, 256], F32R, tag="wo")
            nc.gpsimd.dma_start(out=wo[:, 0, :], in_=wou[0:128, :])
            nc.gpsimd.dma_start(out=wo[:, 1, :], in_=wou[128:256, :])
            rhs_b = const.tile([1, 256], F32R, tag="rhs_b")
            nc.scalar.dma_start(out=rhs_b[:], in_=bou[:].unsqueeze(0))
            ones1 = const.tile([1, 8], F32R, tag="ones1")
            nc.vector.memset(ones1[:].bitcast(F32), 1.0)

            # deferred weight prep: FOLDED decoder kernels (fills the PE
            # bubble while the NTM chain runs).  conv3/conv4 consume a 2x
            # nearest-neighbor-upsampled input, so conv(up2(X)) collapses
            # into 4 output-phase convolutions with 2x2 kernels over the
            # un-upsampled X: phase (a,b) kernel (ey,ex) = sum of W[dy,dx]
            # over dy in GRP[a][ey], dx in GRP[b][ex].  The tap sums are
            # accumulated directly in PSUM by the transposes.
            GRP = (((0,), (1, 2)), ((0, 1), (2,)))
            wfold = {}

            def build_wfold(name, wsrc):
                wf = const.tile([128, 16, 128], B16, tag=f"wfold_{name}")
                nc.vector.memset(wf[:], 0.0)
                for a in range(2):
                    for b in range(2):
                        for ey in range(2):
                            for ex in range(2):
                                k = 8 * a + 4 * b + 2 * ey + ex
                                taps = [
                                    3 * dy + dx
                                    for dy in GRP[a][ey]
                                    for dx in GRP[b][ex]
                                ]
                                pw = psmm.tile([64, 64], F32, tag="mm")
                                for i, t in enumerate(taps):
                                    nc.tensor.matmul(
                                        pw[:],
                                        wsrc[:, t::9],
                                        ident[0:64, 0:64],
                                        is_transpose=True,
                                        start=(i == 0),
                                        stop=(i == len(taps) - 1),
                                    )
                                nc.scalar.activation(
                                    wf[0:64, k, 0:64], pw[:], AF.Copy,
                                    bias=0.0, scale=1.0,
                                )
                nc.sync.dma_start(out=wf[64:128, :, 64:128], in_=wf[0:64, :, 0:64])
                wfold[name] = wf

            wsrc_c3 = load_wsrc(wc3)
            build_wfold("c3", wsrc_c3)
            wsrc_c4 = load_wsrc(wc4)
            build_wfold("c4", wsrc_c4)
            # bridge the gap while the enc evictions land in xstage
            warm(5)

            # ================ NTM step (all 8 samples at once) ==============
            if debug:
                nc.sync.dma_start(out=dbg["dbg_x"][:], in_=xstage[:])
            # x^T k-tiles via PE transpose
            xT = work.tile([128, 2, 8], F32R, tag="xT")
            for kt in range(2):
                pxt = psmm.tile([128, 8], F32, tag="mm")
                nc.tensor.transpose(
                    pxt[:],
                    xstage[:].rearrange("p a b -> p (a b)")[:, kt * 128 : kt * 128 + 128],
                    ident[0:8, 0:8],
                )
                nc.scalar.activation(xT[:, kt, :], pxt[:], AF.Copy, bias=0.0, scale=1.0)
            # z = x @ Wx + b for gates i, g, o; h = sig(o) * tanh(sig(i)*tanh(g))
            zps = psmm.tile([128, 6, 8], F32, tag="mm")
            for j in range(3):
                for h2 in range(2):
                    for kt in range(2):
                        nc.tensor.matmul(
                            zps[:, 2 * j + h2, :],
                            wx[:, kt, j * 256 + h2 * 128 : j * 256 + h2 * 128 + 128],
                            xT[:, kt, :],
                            start=(kt == 0),
                            stop=(kt == 1),
                        )
            zb = work.tile([128, 6, 8], F32, tag="zb")
            bigo_b = bass.AP(
                tensor=bigo[:].tensor, offset=bigo[:].offset,
                ap=[list(d) for d in bigo[:].ap] + [[0, 8]],
            )
            nc.vector.tensor_tensor(zb[:], zps[:], bigo_b, op=ALU.add)
            si = work.tile([128, 2, 8], F32, tag="gate0")
            nc.scalar.activation(si[:], zb[:, 0:2, :], AF.Sigmoid, bias=0.0)
            tg = work.tile([128, 2, 8], F32, tag="gate1")
            nc.scalar.activation(tg[:], zb[:, 2:4, :], AF.Tanh, bias=0.0)
            so = work.tile([128, 2, 8], F32, tag="gate2")
            nc.scalar.activation(so[:], zb[:, 4:6, :], AF.Sigmoid, bias=0.0)
            ctile = work.tile([128, 2, 8], F32, tag="ctile")
            nc.vector.tensor_mul(ctile[:], si[:], tg[:])
            tct = work.tile([128, 2, 8], F32, tag="tct")
            nc.scalar.activation(tct[:], ctile[:], AF.Tanh, bias=0.0)
            h = work.tile([128, 2, 8], F32R, tag="h")
            nc.vector.tensor_mul(h[:], so[:], tct[:])
            if debug:
                nc.sync.dma_start(out=dbg["dbg_h"][:], in_=h[:])
            # out = clip(h @ w_out[:256] + b_out)  (reads contribution dropped)
            pout = psmm.tile([8, 16, 16], F32, tag="mm")
            for kt in range(2):
                nc.tensor.matmul(
                    pout[:].rearrange("p a b -> p (a b)"),
                    h[:, kt, :],
                    wo[:, kt, :],
                    start=(kt == 0),
                    stop=False,
                )
            nc.tensor.matmul(
                pout[:].rearrange("p a b -> p (a b)"),
                ones1[:],
                rhs_b[:],
                start=False,
                stop=True,
            )
            nc.vector.tensor_scalar(
                stg2[:, 1:17, 1:17], pout[:], -CLIP, CLIP, ALU.max, ALU.min
            )
            # keep the PE clock warm while the NTM result fans out through
            # stg2 -> pc2 staging DMAs: the HAM gate needs a DENSE ~3.4us
            # busy window to hold/raise 2.4GHz, and a cold decoder start
            # costs ~15us.
            warm(18)
            if debug:
                nc.sync.dma_start(out=dbg["dbg_clip"][:], in_=stg2[:, 1:17, 1:17])

            # ================ decoder: stage-major over 4 pairs =============
            # conv2 patches for all pairs in one merged tile + one DMA.
            # partition r = 32p + 9s + 3dy + dx via the overlapping stride-1
            # dx trick (reads stg2 shifted by 0/1/2 columns).
            pc2 = const.tile([128, 684], F32R, tag="pc2")
            for p in range(NPAIR):
                base = 32 * p if p < 3 else 0
                c0 = 0 if p < 3 else 342
                for s in range(2):
                    for dy in range(3):
                        eng = dmaeng[(2 * p + s + dy) % 3]
                        eng.dma_start(
                            out=bass.AP(
                                tensor=pc2[:].tensor,
                                offset=pc2[:].offset
                                + (base + 9 * s + 3 * dy) * 684 + c0,
                                ap=[[684, 3], [1, 341]],
                            ),
                            in_=bass.AP(
                                tensor=stg2[:].tensor,
                                offset=stg2[:].offset + (2 * p + s) * 399 + dy * 19,
                                ap=[[399, 1], [1, 3], [1, 341]],
                            ),
                        )

            # --- conv2 all pairs -> padded stage S2 (reuses the quad pool
            # slots freed by the enc pairs; 1 eviction per pair instead of
            # the old 4 upsample writes)
            c2s_l = []
            for p in range(NPAIR):
                base = 32 * p if p < 3 else 0
                c0 = 0 if p < 3 else 342
                ps2 = psmm.tile([128, 16, 16], F32, tag="mm")
                nc.tensor.matmul(
                    ps2[:],
                    ct2_c2[base : base + 18, :],
                    pc2[base : base + 18, c0 : c0 + 342]
                    .rearrange("p (a b) -> p a b", a=18)[:, 0:16, 0:16],
                    start=True,
                    stop=True,
                )
                S2 = quad.tile([128, 18, 18], B16, tag="c2s")
                nc.gpsimd.memset(S2[:, 0:1, :], 0.0)
                nc.gpsimd.memset(S2[:, 17:18, :], 0.0)
                nc.gpsimd.memset(S2[:, 1:17, 0:1], 0.0)
                nc.gpsimd.memset(S2[:, 1:17, 17:18], 0.0)
                nc.scalar.activation(S2[:, 1:17, 1:17], ps2[:], AF.Relu, bias=bt2)
                c2s_l.append(S2)
            if debug:
                nc.sync.dma_start(out=dbg["dbg_pc2"][:], in_=pc2[:])
                nc.sync.dma_start(out=dbg["dbg_ctc2"][:], in_=ct2_c2[:])

            # --- conv3 folded: 4 output-phase 2x2 convs over S2 -> S3
            # (34x34 padded, UN-upsampled conv3 output)
            c3s_l = [None] * NPAIR

            def conv3_pair(p):
                S2 = c2s_l[p]
                S3 = c3p.tile([128, 34, 34], B16, tag="c3s")
                nc.gpsimd.memset(S3[:, 0:1, :], 0.0)
                nc.gpsimd.memset(S3[:, 33:34, :], 0.0)
                nc.gpsimd.memset(S3[:, 1:33, 0:1], 0.0)
                nc.gpsimd.memset(S3[:, 1:33, 33:34], 0.0)
                S3v = S3[:].rearrange(
                    "p (ri ra) (ci cb) -> p ri ra ci cb", ra=2, cb=2
                )
                for a in range(2):
                    for b in range(2):
                        ps = psc3.tile([128, 16, 16], F32, tag="mm3")
                        i = 0
                        for ey in range(2):
                            for ex in range(2):
                                k = 8 * a + 4 * b + 2 * ey + ex
                                nc.tensor.matmul(
                                    ps[:],
                                    wfold["c3"][:, k, :],
                                    S2[:, a + ey : a + ey + 16,
                                       b + ex : b + ex + 16],
                                    start=(i == 0),
                                    stop=(i == 3),
                                )
                                i += 1
                        # out row 1+2r+a, col 1+2c+b in S3
                        rs = slice(0, 16) if a == 0 else slice(1, 17)
                        cs = slice(0, 16) if b == 0 else slice(1, 17)
                        dst = S3v[:, rs, 1 - a, cs, 1 - b]
                        if (a + b) % 2 == 0:
                            nc.scalar.activation(dst, ps[:], AF.Relu, bias=bt3)
                        else:
                            nc.vector.tensor_scalar(
                                dst, ps[:], bt3[:], 0.0, ALU.add, ALU.max
                            )
                c3s_l[p] = S3

            # --- conv4 folded: phases over S3; each 32-row block stores as
            # one 1MB DMA as soon as its 4 phases are evicted.
            store_rot = [nc.gpsimd, nc.sync]

            def conv4_pair(p):
                S3 = c3s_l[p]
                c4out = out2.tile([128, 64, 64], F32, tag="c4out")
                c4v = c4out[:].rearrange(
                    "p (ri ra) (ci cb) -> p ri ra ci cb", ra=2, cb=2
                )
                for h in range(2):
                    for a in range(2):
                        for b in range(2):
                            ps = psmm.tile([128, 16, 32], F32, tag="mm")
                            i = 0
                            for ey in range(2):
                                for ex in range(2):
                                    k = 8 * a + 4 * b + 2 * ey + ex
                                    nc.tensor.matmul(
                                        ps[:],
                                        wfold["c4"][:, k, :],
                                        S3[:, 16 * h + a + ey : 16 * h + a + ey + 16,
                                           b + ex : b + ex + 32],
                                        start=(i == 0),
                                        stop=(i == 3),
                                    )
                                    i += 1
                            dst = c4v[:, 16 * h : 16 * h + 16, a, 0:32, b]
                            if (a + b) % 2 == 0:
                                nc.scalar.activation(dst, ps[:], AF.Relu, bias=bt4)
                            else:
                                nc.vector.tensor_scalar(
                                    dst, ps[:], bt4[:], 0.0, ALU.add, ALU.max
                                )
                    store_rot[(2 * p + h) % 2].dma_start(
                        out=bass.AP(
                            tensor=out[:].tensor,
                            offset=2 * p * 262144 + 2048 * h,
                            ap=[[4096, 128], [1, 2048]],
                        ),
                        in_=bass.AP(
                            tensor=c4out[:].tensor,
                            offset=c4out[:].offset + 2048 * h,
                            ap=[[4096, 128], [1, 2048]],
                        ),
                    )

            conv3_pair(0)
            conv3_pair(1)
            conv4_pair(0)
            conv3_pair(2)
            conv4_pair(1)
            conv3_pair(3)
            conv4_pair(2)
            conv4_pair(3)

    nc.compile()
    return nc


_NC_CACHE = {}
LAST_RESULT = None

WEIGHT_NAMES = [
    "w_conv0", "b_conv0", "w_conv1", "b_conv1", "w_enc", "b_enc",
    "w_conv2", "b_conv2", "w_conv3", "b_conv3", "w_conv4", "b_conv4",
    "w_lstm_x", "b_lstm", "w_out", "b_out",
]


def kernel(**inputs):
    global LAST_RESULT
    from concourse.bass_utils import run_bass_kernel_spmd

    debug = bool(int(os.environ.get("KDEBUG", "0")))
    key = ("nc", debug)
    if key not in _NC_CACHE:
        _NC_CACHE[key] = build_nc(debug=debug)
    nc = _NC_CACHE[key]

    xs = np.ascontiguousarray(np.asarray(inputs["inputs"], dtype=np.float32))
    weights = {
        k: np.ascontiguousarray(np.asarray(inputs[k], dtype=np.float32))
        for k in WEIGHT_NAMES
    }
    in_maps = []
    for c in range(N_CORES):
        m = dict(weights)
        m["inputs"] = xs[c * B_CORE : (c + 1) * B_CORE]
        in_maps.append(m)

    res = run_bass_kernel_spmd(nc, in_maps, core_ids=list(range(N_CORES)))
    LAST_RESULT = res
    return np.concatenate([r["out"] for r in res.results], axis=0)


if __name__ == "__main__":
    nc = build_nc()
    print("built ok")

